# revision 18
# baseline (speedup 1.0000x reference)
"""GAT layer (multi-head graph attention) on 8 TRN2 NeuronCores.

Strategy (per sharding hint): destination nodes are sharded across the 8
cores.  Each core:
  phase 1: computes the full projection table redundantly (bf16 GEMM
           X @ W.T plus the per-head attention score reductions), packed
           as [proj bf16 | s_src f32 | s_tgt f32 | pad] rows in local HBM.
  phase 2: walks its shard's destination windows (128 targets / window).
           Edges are pre-sorted by (window, src-bucket) on the host;
           dma_gather pulls the source rows (int16 indices per 32768-row
           bucket), scores -> leaky-relu -> exp run batched per window,
           and one-hot matmuls (host-streamed) accumulate both the
           softmax denominator and the weighted aggregation in PSUM.
           Softmax division + PReLU happen once per window at flush.

kernel(**inputs) takes the FULL inputs and returns the FULL output.
"""

import math
from dataclasses import dataclass, field

import numpy as np
import ml_dtypes

BF16 = ml_dtypes.bfloat16
P = 128


def _ceil(a, b):
    return -(-a // b)


@dataclass
class Cfg:
    N: int = 100000
    E: int = 800000
    HID: int = 512
    HEADS: int = 8
    ncores: int = 8
    bucket: int = 32768
    leak: float = 0.01
    oh_bf16: bool = True  # one-hot stream dtype (bf16; fp8 is an option)
    GW: int = 4           # windows per gather group
    CH: int = 16          # tiles per phase-2 compute chunk
    p1_split: bool = True  # split-GEMM phase 1 (interleaved psA/psB)

    def __post_init__(self):
        assert self.N % self.ncores == 0
        assert self.bucket <= 32768
        self.F = self.HID // self.HEADS
        self.shard = self.N // self.ncores
        self.NW = _ceil(self.shard, P)          # windows per core
        self.NB = _ceil(self.N, self.bucket)    # src buckets (int16 range)
        self.NT = _ceil(self.N, P)              # projection tiles
        self.NPAD = self.NT * P
        self.KP = min(self.HID, P)              # contraction partitions
        self.KT = self.HID // self.KP           # contraction tiles
        row_bytes = self.HID * 2 + 2 * self.HEADS * 4
        self.row_used = row_bytes               # bytes actually written
        self.row_bytes = _ceil(row_bytes, 256) * 256
        self.row_bf = self.row_bytes // 2
        self.row_f32 = self.row_bytes // 4
        self.s_src_off = self.HID // 2          # f32 col of s_src in a row
        self.s_tgt_off = self.HID // 2 + self.HEADS
        # phase-1 output split: colsA covers proj[0:splitA] plus the 2H score
        # columns; colsB covers proj[splitA:HID].  Streams are balanced so
        # every LDWEIGHTS hides under the previous matmul's column stream.
        self.splitA = self.HID // 2 - self.HEADS * 2  # 240
        self.colsA = self.splitA + 2 * self.HEADS     # 256
        self.colsB = self.HID - self.splitA           # 272


@dataclass
class Schedule:
    """Core-independent (uniform) phase-2 schedule.

    Windows are processed in groups of GW; each gather call covers one
    (group, bucket) pair so the ~4us fixed per-call GpSimd cost is paid
    ~NW/GW*NB times instead of NW*NB times.  Within a group slots are laid
    out bucket-major: [b0: w0|w1|w2|w3, b1: w0|..., ...], each (w,b)
    segment padded to a 128 multiple so no tile mixes windows.
    """
    seg: np.ndarray          # [NW, NB] slot counts (128-aligned, global max)
    groups: list             # list of lists of window indices
    TG: list                 # tiles per group
    TGmax: int
    calls: list              # per group: list of (b, slot_off, nslots, idxcol0)
    seg_off: dict            # (w, b) -> slot offset within its group
    tile_w: list             # per group: window index (in-group) per tile
    win_last_tile: list      # per group: in-group last tile idx per window
    idxcols: int             # total int16 idx columns (per 16-wrap row)
    TT: int                  # total tiles
    tile_base: list          # first global tile index of each group


def build_schedule(cfg: Cfg, counts: np.ndarray) -> Schedule:
    """counts: [ncores, NW, NB] edge counts."""
    maxcnt = counts.max(axis=0)  # [NW, NB]
    seg = np.where(maxcnt > 0, _ceil(maxcnt, P) * P, 0).astype(np.int64)
    groups = [list(range(g0, min(g0 + cfg.GW, cfg.NW)))
              for g0 in range(0, cfg.NW, cfg.GW)]
    TG, calls, tile_base, tile_w, win_last_tile = [], [], [], [], []
    seg_off = {}
    idxcol = 0
    tt = 0
    for grp in groups:
        tile_base.append(tt)
        gcalls = []
        tw = []
        last = {wi: -1 for wi in range(len(grp))}
        off = 0
        for b in range(cfg.NB):
            nslots = int(sum(seg[w, b] for w in grp))
            if nslots == 0:
                continue
            gcalls.append((b, off, nslots, idxcol))
            for wi, w in enumerate(grp):
                s = int(seg[w, b])
                if s == 0:
                    continue
                seg_off[(w, b)] = off
                for _ in range(s // P):
                    last[wi] = len(tw)
                    tw.append(wi)
                off += s
            idxcol += nslots // 16
        assert off % P == 0
        TG.append(off // P)
        tt += off // P
        calls.append(gcalls)
        tile_w.append(tw)
        win_last_tile.append(last)
    return Schedule(seg=seg, groups=groups, TG=TG, TGmax=max(TG), calls=calls,
                    seg_off=seg_off, tile_w=tile_w, win_last_tile=win_last_tile,
                    idxcols=idxcol, TT=tt, tile_base=tile_base)


def prep_core(cfg: Cfg, sched: Schedule, src, trg, k):
    """Per-core input arrays: g1 idx stream and one-hot stream.

    Padding slots inside a call (between window segments) get index 0 (the
    bucket's first row: real, finite data, zero one-hot columns).  Padding
    at the very end of a call gets -1, which the gather ucode trims so it
    costs neither descriptors nor HBM reads.
    """
    oh_dt = BF16 if cfg.oh_bf16 else ml_dtypes.float8_e4m3
    mask = (trg // cfg.shard) == k
    esrc = src[mask]
    etrg = trg[mask]
    trel = etrg - k * cfg.shard
    win = trel // P
    buck = esrc // cfg.bucket
    # order edges by (window, bucket); stable so host/device agree
    order = np.lexsort((buck, win))
    esrc, etrg, trel, win, buck = (a[order] for a in (esrc, etrg, trel, win, buck))

    g1i = np.zeros((P, sched.idxcols), np.int16)
    oh = np.zeros((P, sched.TT, 2, P), oh_dt)

    # per (window, bucket) segment boundaries
    key = win * cfg.NB + buck
    starts = np.searchsorted(key, np.arange(cfg.NW * cfg.NB), side="left")
    ends = np.searchsorted(key, np.arange(cfg.NW * cfg.NB), side="right")

    for g, grp in enumerate(sched.groups):
        for (b, call_off, nslots, idxcol0) in sched.calls[g]:
            idx = np.zeros(nslots, np.int16)
            pos = 0
            pad_last = 0
            for w in grp:
                s = int(sched.seg[w, b])
                if s == 0:
                    continue
                lo, hi = int(starts[w * cfg.NB + b]), int(ends[w * cfg.NB + b])
                cnt = hi - lo
                assert cnt <= s
                idx[pos:pos + cnt] = (esrc[lo:hi] - b * cfg.bucket).astype(np.int16)
                # idx[pos+cnt : pos+s] stays 0 (mid-call padding)
                pad_last = s - cnt
                # one-hots for this segment's slots
                tloc = (trel[lo:hi] - w * P).astype(np.int64)   # [cnt] in [0,128)
                gslot = call_off + pos + np.arange(cnt)
                tgl = sched.tile_base[g] + gslot // P
                oh[gslot % P, tgl, 0, tloc] = oh_dt(1.0)
                oh[tloc, tgl, 1, gslot % P] = oh_dt(1.0)
                pos += s
            assert pos == nslots
            del pad_last  # all padding gathers bucket row 0 (finite, oh cols 0)
            blk = idx.reshape(nslots // 16, 16).T          # [16, cols]
            g1i[:, idxcol0:idxcol0 + nslots // 16] = np.tile(blk, (8, 1))
    return g1i, oh


def pack_xt(cfg: Cfg, X: np.ndarray) -> np.ndarray:
    """X [N, HID] f32 -> bf16 packed [KP, NT, KT, P]: (p, j, ki, n) = X[j*P+n, ki*KP+p]."""
    Xp = np.zeros((cfg.NPAD, cfg.HID), np.float32)
    Xp[: cfg.N] = X
    Xb = Xp.astype(BF16)
    # [NT, P(n), KT, KP(p)] -> transpose to [KP, NT, KT, P]
    v = Xb.reshape(cfg.NT, P, cfg.KT, cfg.KP)
    return np.ascontiguousarray(v.transpose(3, 0, 2, 1))


def pack_w(cfg: Cfg, W, a_src, a_tgt):
    """Returns wA [KP, KT, colsA] and wB [KP, KT, colsB] bf16.

    wA = [W.T[:, :splitA] | WA] (proj prefix plus both score projections),
    wB = W.T[:, splitA:].  Splitting the 528 output columns into two
    balanced streams lets every LDWEIGHTS hide under a matmul stream.
    """
    WT = W.T.astype(np.float32)                       # [HID(d), HID(o)]
    wa_s = (W.reshape(cfg.HEADS, cfg.F, cfg.HID)
            * np.asarray(a_src, np.float32).reshape(cfg.HEADS, cfg.F, 1)).sum(1)  # [H, d]
    wa_t = (W.reshape(cfg.HEADS, cfg.F, cfg.HID)
            * np.asarray(a_tgt, np.float32).reshape(cfg.HEADS, cfg.F, 1)).sum(1)
    WA = np.concatenate([wa_s.T, wa_t.T], axis=1)     # [d, 2H]
    if not cfg.p1_split:
        wAf, wBf = WT, WA      # original layout: full W.T stream + score stream
    else:
        wAf = np.concatenate([WT[:, :cfg.splitA], WA], axis=1)   # [d, colsA]
        wBf = WT[:, cfg.splitA:]                                  # [d, colsB]
    wA = np.ascontiguousarray(
        wAf.astype(BF16).reshape(cfg.KT, cfg.KP, wAf.shape[1]).transpose(1, 0, 2))
    wB = np.ascontiguousarray(
        wBf.astype(BF16).reshape(cfg.KT, cfg.KP, wBf.shape[1]).transpose(1, 0, 2))
    return wA, wB


def _bcast_last(ap, n):
    """Append a 0-stride broadcast dim of size n to an AP."""
    import concourse.bass as bass
    lst = [list(x) for x in ap.ap] + [[0, n]]
    return bass.AP(ap.tensor, ap.offset, lst)


def build_nc(cfg: Cfg, sched: Schedule, phases: str = "full"):
    import concourse.bacc as bacc
    import concourse.bass as bass
    import concourse.mybir as mybir
    from concourse.tile import TileContext

    dt = mybir.dt
    oh_mdt = dt.bfloat16 if cfg.oh_bf16 else dt.float8e4
    H, HID, KT, KP = cfg.HEADS, cfg.HID, cfg.KT, cfg.KP

    nc = bacc.Bacc("TRN2", target_bir_lowering=False)

    cA = cfg.colsA if cfg.p1_split else HID
    cB = cfg.colsB if cfg.p1_split else 2 * H
    xt = nc.dram_tensor("xt", [KP, cfg.NT, KT, P], dt.bfloat16, kind="ExternalInput")
    wt = nc.dram_tensor("wt", [KP, KT, cA], dt.bfloat16, kind="ExternalInput")
    wa = nc.dram_tensor("wa", [KP, KT, cB], dt.bfloat16, kind="ExternalInput")
    g1i = nc.dram_tensor("g1i", [P, sched.idxcols], dt.int16, kind="ExternalInput")
    ohd = nc.dram_tensor("ohd", [P, sched.TT, 2, P], oh_mdt, kind="ExternalInput")
    avec = nc.dram_tensor("avec", [P, 1], dt.float32, kind="ExternalInput")
    out = nc.dram_tensor("out", [cfg.NW * P, HID], dt.float32, kind="ExternalOutput")

    with TileContext(nc) as tc:
        with tc.tile_pool(name="const", bufs=1) as cpool, \
             tc.tile_pool(name="dram", bufs=1, space="DRAM") as dpool:
            table = dpool.tile([cfg.NPAD, cfg.row_bf], dt.bfloat16)
            wt_sb = cpool.tile([KP, KT, cA], dt.bfloat16)
            nc.sync.dma_start(out=wt_sb[:], in_=wt[:, :, :])
            wa_sb = cpool.tile([KP, KT, cB], dt.bfloat16)
            nc.sync.dma_start(out=wa_sb[:], in_=wa[:, :, :])
            if phases == "full":
                a_sb = cpool.tile([P, 1], dt.float32)
                nc.sync.dma_start(out=a_sb[:], in_=avec[:, :])
            if phases in ("full", "p1g"):
                g1i_sb = cpool.tile([P, sched.idxcols], dt.int16)
                nc.sync.dma_start(out=g1i_sb[:], in_=g1i[:, :])

            # ---------------- phase 1: projection table ----------------
            # Two balanced column streams (colsA=256 incl. the 16 score cols,
            # colsB=272) with interleaved matmuls so LDWEIGHTS always hides
            # under the previous stream.  Rows are written 1088B of 1280B
            # (the 192B tail is never read).
            used_bf = cfg.row_used // 2              # 544 bf16 per row
            with tc.tile_pool(name="p1", bufs=3) as xpool, \
                 tc.tile_pool(name="p1ps", bufs=2, space="PSUM") as pspool, \
                 tc.tile_pool(name="p1st", bufs=3) as stpool:
                for j in range(cfg.NT):
                    xtile = xpool.tile([KP, KT, P], dt.bfloat16, tag="x")
                    nc.sync.dma_start(out=xtile[:], in_=xt[:, j, :, :])
                    # full-bank tiles: a matmul output must stay inside one
                    # 2KB PSUM bank, and concurrent accumulation groups must
                    # live in different banks.
                    psA_f = pspool.tile([P, 512], dt.float32, tag="psA")
                    psB_f = pspool.tile([P, 512], dt.float32, tag="psB")
                    psA = psA_f[:, 0:cA]
                    psB = psB_f[:, 0:cB]
                    if cfg.p1_split:
                        for ki in range(KT):
                            nc.tensor.matmul(psA[:], xtile[:, ki, :], wt_sb[:, ki, :],
                                             start=(ki == 0), stop=(ki == KT - 1))
                            nc.tensor.matmul(psB[:], xtile[:, ki, :], wa_sb[:, ki, :],
                                             start=(ki == 0), stop=(ki == KT - 1))
                    else:
                        for ki in range(KT):
                            nc.tensor.matmul(psA[:], xtile[:, ki, :], wt_sb[:, ki, :],
                                             start=(ki == 0), stop=(ki == KT - 1))
                        for ki in range(KT):
                            nc.tensor.matmul(psB[:], xtile[:, ki, :], wa_sb[:, ki, :],
                                             start=(ki == 0), stop=(ki == KT - 1))
                    stg = stpool.tile([P, used_bf], dt.bfloat16, tag="stg")
                    stg32 = stg.bitcast(dt.float32)
                    if cfg.p1_split:
                        nc.scalar.copy(out=stg[:, 0:cfg.splitA],
                                       in_=psA[:, 0:cfg.splitA])
                        nc.scalar.copy(
                            out=stg32[:, cfg.s_src_off:cfg.s_src_off + 2 * H],
                            in_=psA[:, cfg.splitA:cfg.colsA])
                        nc.vector.tensor_copy(out=stg[:, cfg.splitA:HID], in_=psB[:])
                    else:
                        nc.scalar.copy(out=stg[:, 0:HID], in_=psA[:])
                        nc.scalar.copy(
                            out=stg32[:, cfg.s_src_off:cfg.s_src_off + 2 * H],
                            in_=psB[:])
                    nc.sync.dma_start(
                        out=table[j * P:(j + 1) * P, 0:used_bf], in_=stg[:])

            tc.strict_bb_all_engine_barrier()

            # ---------------- phase 1.5: resident s_tgt (hi/lo bf16) ----------------
            pid = nc.sync.partition_id()
            table32 = table.bitcast(dt.float32)
            s_ap = table32[bass.DynSlice(pid * cfg.shard, cfg.NW * P),
                           cfg.s_tgt_off:cfg.s_tgt_off + H]
            s_ap = s_ap.rearrange("(w p) h -> p w h", p=P)
            s_all = cpool.tile([P, cfg.NW, H], dt.float32)
            nc.sync.dma_start(out=s_all[:], in_=s_ap)
            s_hilo = cpool.tile([P, cfg.NW, 2, H], dt.bfloat16)
            s_hi32 = cpool.tile([P, cfg.NW, H], dt.float32)
            nc.vector.tensor_copy(out=s_hilo[:, :, 0, :], in_=s_all[:])
            nc.vector.tensor_copy(out=s_hi32[:], in_=s_hilo[:, :, 0, :])
            nc.vector.tensor_tensor(out=s_hilo[:, :, 1, :], in0=s_all[:],
                                    in1=s_hi32[:], op=mybir.AluOpType.subtract)

            # ---------------- phase 2: window groups ----------------
            CH = cfg.CH
            with tc.tile_pool(name="p2", bufs=2) as pool, \
                 tc.tile_pool(name="p2c", bufs=2) as cpool2, \
                 tc.tile_pool(name="p2ps", bufs=2, space="PSUM") as pps, \
                 tc.tile_pool(name="p2acc", bufs=1, space="PSUM") as apool:
                # Zero both g1t rotation buffers once: slots whose gather was
                # trimmed (trailing -1 indices) read stale SBUF, which must be
                # finite.  After the first two groups, stale bytes are old
                # gathered rows (finite bf16/f32), so one round suffices.
                for _ in range(2):
                    g1z = pool.tile([P, sched.TGmax, cfg.row_bf], dt.bfloat16,
                                    tag="g1t")
                    nc.vector.memset(g1z[:], 0.0)
                for g, grp in enumerate(sched.groups):
                    Tg = sched.TG[g]
                    nw = len(grp)
                    g1t = pool.tile([P, sched.TGmax, cfg.row_bf], dt.bfloat16,
                                    tag="g1t")
                    for (b, slot_off, nslots, idxcol0) in sched.calls[g]:
                        rows = min(cfg.NPAD, (b + 1) * cfg.bucket) - b * cfg.bucket
                        # single_packet chains the call's descriptors into
                        # one SDMA packet; the HW packet limit is 64
                        # descriptors, so large merged calls must split.
                        nc.gpsimd.dma_gather(
                            g1t[:, slot_off // P:(slot_off + nslots) // P, :],
                            table[b * cfg.bucket:b * cfg.bucket + rows, :],
                            g1i_sb[:, idxcol0:idxcol0 + nslots // 16],
                            nslots, nslots, cfg.row_bf,
                            single_packet=(nslots // 16 + 1 <= 64))
                    g1t32 = g1t.bitcast(dt.float32)
                    jb = sched.tile_base[g]
                    agg = apool.tile([P, cfg.GW, HID], dt.float32, tag="agg")
                    den = apool.tile([P, cfg.GW, H], dt.float32, tag="den")
                    den_acc = pool.tile([P, cfg.GW, H], dt.float32, tag="den_acc")
                    den_seen = set()
                    started = [False] * nw
                    for c in range(_ceil(Tg, CH)):
                        t0, t1 = c * CH, min(Tg, (c + 1) * CH)
                        tn = t1 - t0
                        ohc = cpool2.tile([P, CH, 2, P], oh_mdt, tag="ohc")
                        nc.sync.dma_start(out=ohc[:, :tn, :, :],
                                          in_=ohd[:, jb + t0:jb + t1, :, :])
                        stgt = pps.tile([P, CH, 2, H], dt.float32, tag="stgt")
                        for t in range(t0, t1):
                            nc.tensor.matmul(
                                stgt[:, t - t0, :, :], ohc[:, t - t0, 1, :],
                                s_hilo[:, grp[sched.tile_w[g][t]], :, :],
                                start=True, stop=True)
                        s_sum = cpool2.tile([P, CH, H], dt.float32, tag="s_sum")
                        s_act = cpool2.tile([P, CH, H], dt.float32, tag="s_act")
                        nc.vector.tensor_tensor(
                            out=s_sum[:, :tn, :], in0=stgt[:, :tn, 0, :],
                            in1=g1t32[:, t0:t1, cfg.s_src_off:cfg.s_src_off + H],
                            op=mybir.AluOpType.add)
                        nc.vector.tensor_tensor(
                            out=s_act[:, :tn, :], in0=stgt[:, :tn, 1, :],
                            in1=s_sum[:, :tn, :], op=mybir.AluOpType.add)
                        nc.vector.scalar_tensor_tensor(
                            out=s_sum[:, :tn, :], in0=s_act[:, :tn, :],
                            scalar=cfg.leak, in1=s_act[:, :tn, :],
                            op0=mybir.AluOpType.mult, op1=mybir.AluOpType.max)
                        exp_t = cpool2.tile([P, CH, H], dt.bfloat16, tag="exp_t")
                        nc.scalar.activation(out=exp_t[:, :tn, :],
                                             in_=s_sum[:, :tn, :],
                                             func=mybir.ActivationFunctionType.Exp)

                        w_t = cpool2.tile([P, CH, HID], dt.bfloat16, tag="w_t")
                        proj4 = g1t[:, t0:t1, 0:HID].rearrange(
                            "p t (h f) -> p t h f", h=H)
                        exp4 = _bcast_last(exp_t[:, :tn, :], cfg.F)
                        out4 = w_t[:, :tn, :].rearrange("p t (h f) -> p t h f", h=H)
                        nc.vector.tensor_tensor(out=out4, in0=proj4, in1=exp4,
                                                op=mybir.AluOpType.mult)

                        # tiles grouped by window: agg chains span the whole
                        # group (each window's agg is its own PSUM bank); den
                        # groups open/close within this chunk (all windows
                        # share one bank, so groups must not interleave), and
                        # chunk partials accumulate into den_acc on DVE.
                        bywin = {}
                        for t in range(t0, t1):
                            bywin.setdefault(sched.tile_w[g][t], []).append(t)
                        for wi, tlist in bywin.items():
                            for t in tlist:
                                first = not started[wi]
                                started[wi] = True
                                lastt = (t == sched.win_last_tile[g][wi])
                                nc.tensor.matmul(agg[:, wi, :],
                                                 ohc[:, t - t0, 0, :],
                                                 w_t[:, t - t0, :],
                                                 start=first, stop=lastt)
                                nc.tensor.matmul(den[:, wi, :],
                                                 ohc[:, t - t0, 0, :],
                                                 exp_t[:, t - t0, :],
                                                 start=(t == tlist[0]),
                                                 stop=(t == tlist[-1]))
                            if wi in den_seen:
                                nc.vector.tensor_tensor(
                                    out=den_acc[:, wi, :], in0=den[:, wi, :],
                                    in1=den_acc[:, wi, :], op=mybir.AluOpType.add)
                            else:
                                nc.vector.tensor_copy(out=den_acc[:, wi, :],
                                                      in_=den[:, wi, :])
                                den_seen.add(wi)

                    # flush: softmax divide + PReLU for all windows in group
                    den_sb = pool.tile([P, cfg.GW, H], dt.float32, tag="den_sb")
                    recip = pool.tile([P, cfg.GW, H], dt.float32, tag="recip")
                    nc.vector.tensor_scalar_add(out=den_sb[:, :nw, :],
                                                in0=den_acc[:, :nw, :],
                                                scalar1=1e-16)
                    nc.vector.reciprocal(out=recip[:, :nw, :], in_=den_sb[:, :nw, :])
                    for wi, w in enumerate(grp):
                        z = pool.tile([P, HID], dt.float32, tag="z")
                        agg4 = agg[:, wi, :].rearrange("p (h f) -> p h f", h=H)
                        z4 = z[:].rearrange("p (h f) -> p h f", h=H)
                        nc.vector.tensor_tensor(
                            out=z4, in0=agg4,
                            in1=_bcast_last(recip[:, wi, :], cfg.F),
                            op=mybir.AluOpType.mult)
                        res = pool.tile([P, HID], dt.float32, tag="res")
                        nc.vector.scalar_tensor_tensor(
                            out=res[:], in0=z[:], scalar=a_sb[:, 0:1], in1=z[:],
                            op0=mybir.AluOpType.mult, op1=mybir.AluOpType.max)
                        nc.sync.dma_start(out=out[w * P:(w + 1) * P, :], in_=res[:])

    nc.compile()
    return nc


def prepare(cfg: Cfg, inputs):
    """Host-side prep shared by HW and sim paths.

    Returns (sched, in_maps, assemble) where assemble(core_outs) -> full out.
    """
    X = np.asarray(inputs["in_nodes_features"], np.float32)
    ei = np.asarray(inputs["edge_index"], np.int64)
    W = np.asarray(inputs["W"], np.float32)
    b_lin = np.asarray(inputs["b_lin"], np.float32)
    a_src = np.asarray(inputs["a_src"], np.float32)
    a_tgt = np.asarray(inputs["a_tgt"], np.float32)
    bias = np.asarray(inputs["bias"], np.float32)
    prelu_a = float(np.asarray(inputs["prelu_a"], np.float32))

    assert np.all(b_lin == 0) and np.all(bias == 0), "nonzero bias unsupported"
    assert 0.0 <= prelu_a <= 1.0, "prelu_a outside [0,1] unsupported"

    src, trg = ei[0], ei[1]
    core_of = trg // cfg.shard
    win_of = (trg % cfg.shard) // P
    buck_of = src // cfg.bucket
    counts = np.zeros((cfg.ncores, cfg.NW, cfg.NB), np.int64)
    for k in range(cfg.ncores):
        m = core_of == k
        counts[k] = np.bincount(
            win_of[m] * cfg.NB + buck_of[m],
            minlength=cfg.NW * cfg.NB).reshape(cfg.NW, cfg.NB)
    sched = build_schedule(cfg, counts)

    xt = pack_xt(cfg, X)
    wtp, wap = pack_w(cfg, W, a_src, a_tgt)
    av = np.full((P, 1), prelu_a, np.float32)

    in_maps = []
    for k in range(cfg.ncores):
        g1i_k, oh_k = prep_core(cfg, sched, src, trg, k)
        in_maps.append({
            "xt": xt, "wt": wtp, "wa": wap,
            "g1i": g1i_k, "ohd": oh_k, "avec": av,
        })

    def assemble(core_outs):
        return np.concatenate(
            [np.asarray(o["out"][: cfg.shard], np.float32) for o in core_outs], axis=0)

    return sched, in_maps, assemble


_BUILT = {}


def _get_built(cfg: Cfg, sched: Schedule):
    key = (cfg.N, cfg.E, cfg.HID, cfg.HEADS, cfg.ncores, cfg.bucket,
           tuple(sched.TW), sched.idxcols)
    if key not in _BUILT:
        _BUILT[key] = build_nc(cfg, sched)
    return _BUILT[key]


def kernel(**inputs):
    from concourse.bass_utils import run_bass_kernel_spmd

    cfg = Cfg()
    sched, in_maps, assemble = prepare(cfg, inputs)
    nc = _get_built(cfg, sched)
    res = run_bass_kernel_spmd(nc, in_maps, core_ids=list(range(cfg.ncores)))
    return assemble(res.results)



# revision 19
# speedup vs baseline: 1.0139x; 1.0139x over previous
"""GAT layer (multi-head graph attention) on 8 TRN2 NeuronCores.

Strategy (per sharding hint): destination nodes are sharded across the 8
cores.  Each core:
  phase 1: computes the full projection table redundantly (bf16 GEMM
           X @ W.T plus the per-head attention score reductions), packed
           as [proj bf16 | s_src f32 | s_tgt f32 | pad] rows in local HBM.
  phase 2: walks its shard's destination windows (128 targets / window).
           Edges are pre-sorted by (window, src-bucket) on the host;
           dma_gather pulls the source rows (int16 indices per 32768-row
           bucket), scores -> leaky-relu -> exp run batched per window,
           and one-hot matmuls (host-streamed) accumulate both the
           softmax denominator and the weighted aggregation in PSUM.
           Softmax division + PReLU happen once per window at flush.

kernel(**inputs) takes the FULL inputs and returns the FULL output.
"""

import math
from dataclasses import dataclass, field

import numpy as np
import ml_dtypes

BF16 = ml_dtypes.bfloat16
P = 128


def _ceil(a, b):
    return -(-a // b)


@dataclass
class Cfg:
    N: int = 100000
    E: int = 800000
    HID: int = 512
    HEADS: int = 8
    ncores: int = 8
    bucket: int = 32768
    leak: float = 0.01
    oh_bf16: bool = True  # one-hot stream dtype (bf16; fp8 is an option)
    GW: int = 1           # windows per gather group
    CH: int = 16          # tiles per phase-2 compute chunk
    p1_split: bool = True  # split-GEMM phase 1 (interleaved psA/psB)

    def __post_init__(self):
        assert self.N % self.ncores == 0
        assert self.bucket <= 32768
        self.F = self.HID // self.HEADS
        self.shard = self.N // self.ncores
        self.NW = _ceil(self.shard, P)          # windows per core
        self.NB = _ceil(self.N, self.bucket)    # src buckets (int16 range)
        self.NT = _ceil(self.N, P)              # projection tiles
        self.NPAD = self.NT * P
        self.KP = min(self.HID, P)              # contraction partitions
        self.KT = self.HID // self.KP           # contraction tiles
        row_bytes = self.HID * 2 + 2 * self.HEADS * 4
        self.row_used = row_bytes               # bytes actually written
        self.row_bytes = _ceil(row_bytes, 256) * 256
        self.row_bf = self.row_bytes // 2
        self.row_f32 = self.row_bytes // 4
        self.s_src_off = self.HID // 2          # f32 col of s_src in a row
        self.s_tgt_off = self.HID // 2 + self.HEADS
        # phase-1 output split: colsA covers proj[0:splitA] plus the 2H score
        # columns; colsB covers proj[splitA:HID].  Streams are balanced so
        # every LDWEIGHTS hides under the previous matmul's column stream.
        self.splitA = self.HID // 2 - self.HEADS * 2  # 240
        self.colsA = self.splitA + 2 * self.HEADS     # 256
        self.colsB = self.HID - self.splitA           # 272


@dataclass
class Schedule:
    """Core-independent (uniform) phase-2 schedule.

    Windows are processed in groups of GW; each gather call covers one
    (group, bucket) pair so the ~4us fixed per-call GpSimd cost is paid
    ~NW/GW*NB times instead of NW*NB times.  Within a group slots are laid
    out bucket-major: [b0: w0|w1|w2|w3, b1: w0|..., ...], each (w,b)
    segment padded to a 128 multiple so no tile mixes windows.
    """
    seg: np.ndarray          # [NW, NB] slot counts (128-aligned, global max)
    groups: list             # list of lists of window indices
    TG: list                 # tiles per group
    TGmax: int
    calls: list              # per group: list of (b, slot_off, nslots, idxcol0)
    seg_off: dict            # (w, b) -> slot offset within its group
    tile_w: list             # per group: window index (in-group) per tile
    win_last_tile: list      # per group: in-group last tile idx per window
    idxcols: int             # total int16 idx columns (per 16-wrap row)
    TT: int                  # total tiles
    tile_base: list          # first global tile index of each group


def build_schedule(cfg: Cfg, counts: np.ndarray) -> Schedule:
    """counts: [ncores, NW, NB] edge counts."""
    maxcnt = counts.max(axis=0)  # [NW, NB]
    seg = np.where(maxcnt > 0, _ceil(maxcnt, P) * P, 0).astype(np.int64)
    groups = [list(range(g0, min(g0 + cfg.GW, cfg.NW)))
              for g0 in range(0, cfg.NW, cfg.GW)]
    TG, calls, tile_base, tile_w, win_last_tile = [], [], [], [], []
    seg_off = {}
    idxcol = 0
    tt = 0
    for grp in groups:
        tile_base.append(tt)
        gcalls = []
        tw = []
        last = {wi: -1 for wi in range(len(grp))}
        off = 0
        for b in range(cfg.NB):
            nslots = int(sum(seg[w, b] for w in grp))
            if nslots == 0:
                continue
            gcalls.append((b, off, nslots, idxcol))
            for wi, w in enumerate(grp):
                s = int(seg[w, b])
                if s == 0:
                    continue
                seg_off[(w, b)] = off
                for _ in range(s // P):
                    last[wi] = len(tw)
                    tw.append(wi)
                off += s
            idxcol += nslots // 16
        assert off % P == 0
        TG.append(off // P)
        tt += off // P
        calls.append(gcalls)
        tile_w.append(tw)
        win_last_tile.append(last)
    return Schedule(seg=seg, groups=groups, TG=TG, TGmax=max(TG), calls=calls,
                    seg_off=seg_off, tile_w=tile_w, win_last_tile=win_last_tile,
                    idxcols=idxcol, TT=tt, tile_base=tile_base)


def prep_core(cfg: Cfg, sched: Schedule, src, trg, k):
    """Per-core input arrays: g1 idx stream and one-hot stream.

    Padding slots inside a call (between window segments) get index 0 (the
    bucket's first row: real, finite data, zero one-hot columns).  Padding
    at the very end of a call gets -1, which the gather ucode trims so it
    costs neither descriptors nor HBM reads.
    """
    oh_dt = BF16 if cfg.oh_bf16 else ml_dtypes.float8_e4m3
    mask = (trg // cfg.shard) == k
    esrc = src[mask]
    etrg = trg[mask]
    trel = etrg - k * cfg.shard
    win = trel // P
    buck = esrc // cfg.bucket
    # order edges by (window, bucket); stable so host/device agree
    order = np.lexsort((buck, win))
    esrc, etrg, trel, win, buck = (a[order] for a in (esrc, etrg, trel, win, buck))

    g1i = np.zeros((P, sched.idxcols), np.int16)
    oh = np.zeros((P, sched.TT, 2, P), oh_dt)

    # per (window, bucket) segment boundaries
    key = win * cfg.NB + buck
    starts = np.searchsorted(key, np.arange(cfg.NW * cfg.NB), side="left")
    ends = np.searchsorted(key, np.arange(cfg.NW * cfg.NB), side="right")

    for g, grp in enumerate(sched.groups):
        for (b, call_off, nslots, idxcol0) in sched.calls[g]:
            idx = np.zeros(nslots, np.int16)
            pos = 0
            pad_last = 0
            for w in grp:
                s = int(sched.seg[w, b])
                if s == 0:
                    continue
                lo, hi = int(starts[w * cfg.NB + b]), int(ends[w * cfg.NB + b])
                cnt = hi - lo
                assert cnt <= s
                idx[pos:pos + cnt] = (esrc[lo:hi] - b * cfg.bucket).astype(np.int16)
                # idx[pos+cnt : pos+s] stays 0 (mid-call padding)
                pad_last = s - cnt
                # one-hots for this segment's slots
                tloc = (trel[lo:hi] - w * P).astype(np.int64)   # [cnt] in [0,128)
                gslot = call_off + pos + np.arange(cnt)
                tgl = sched.tile_base[g] + gslot // P
                oh[gslot % P, tgl, 0, tloc] = oh_dt(1.0)
                oh[tloc, tgl, 1, gslot % P] = oh_dt(1.0)
                pos += s
            assert pos == nslots
            del pad_last  # all padding gathers bucket row 0 (finite, oh cols 0)
            blk = idx.reshape(nslots // 16, 16).T          # [16, cols]
            g1i[:, idxcol0:idxcol0 + nslots // 16] = np.tile(blk, (8, 1))
    return g1i, oh


def pack_xt(cfg: Cfg, X: np.ndarray) -> np.ndarray:
    """X [N, HID] f32 -> bf16 packed [KP, NT, KT, P]: (p, j, ki, n) = X[j*P+n, ki*KP+p]."""
    Xp = np.zeros((cfg.NPAD, cfg.HID), np.float32)
    Xp[: cfg.N] = X
    Xb = Xp.astype(BF16)
    # [NT, P(n), KT, KP(p)] -> transpose to [KP, NT, KT, P]
    v = Xb.reshape(cfg.NT, P, cfg.KT, cfg.KP)
    return np.ascontiguousarray(v.transpose(3, 0, 2, 1))


def pack_w(cfg: Cfg, W, a_src, a_tgt):
    """Returns wA [KP, KT, colsA] and wB [KP, KT, colsB] bf16.

    wA = [W.T[:, :splitA] | WA] (proj prefix plus both score projections),
    wB = W.T[:, splitA:].  Splitting the 528 output columns into two
    balanced streams lets every LDWEIGHTS hide under a matmul stream.
    """
    WT = W.T.astype(np.float32)                       # [HID(d), HID(o)]
    wa_s = (W.reshape(cfg.HEADS, cfg.F, cfg.HID)
            * np.asarray(a_src, np.float32).reshape(cfg.HEADS, cfg.F, 1)).sum(1)  # [H, d]
    wa_t = (W.reshape(cfg.HEADS, cfg.F, cfg.HID)
            * np.asarray(a_tgt, np.float32).reshape(cfg.HEADS, cfg.F, 1)).sum(1)
    WA = np.concatenate([wa_s.T, wa_t.T], axis=1)     # [d, 2H]
    if not cfg.p1_split:
        wAf, wBf = WT, WA      # original layout: full W.T stream + score stream
    else:
        wAf = np.concatenate([WT[:, :cfg.splitA], WA], axis=1)   # [d, colsA]
        wBf = WT[:, cfg.splitA:]                                  # [d, colsB]
    wA = np.ascontiguousarray(
        wAf.astype(BF16).reshape(cfg.KT, cfg.KP, wAf.shape[1]).transpose(1, 0, 2))
    wB = np.ascontiguousarray(
        wBf.astype(BF16).reshape(cfg.KT, cfg.KP, wBf.shape[1]).transpose(1, 0, 2))
    return wA, wB


def _bcast_last(ap, n):
    """Append a 0-stride broadcast dim of size n to an AP."""
    import concourse.bass as bass
    lst = [list(x) for x in ap.ap] + [[0, n]]
    return bass.AP(ap.tensor, ap.offset, lst)


def build_nc(cfg: Cfg, sched: Schedule, phases: str = "full"):
    import concourse.bacc as bacc
    import concourse.bass as bass
    import concourse.mybir as mybir
    from concourse.tile import TileContext

    dt = mybir.dt
    oh_mdt = dt.bfloat16 if cfg.oh_bf16 else dt.float8e4
    H, HID, KT, KP = cfg.HEADS, cfg.HID, cfg.KT, cfg.KP

    nc = bacc.Bacc("TRN2", target_bir_lowering=False)

    cA = cfg.colsA if cfg.p1_split else HID
    cB = cfg.colsB if cfg.p1_split else 2 * H
    xt = nc.dram_tensor("xt", [KP, cfg.NT, KT, P], dt.bfloat16, kind="ExternalInput")
    wt = nc.dram_tensor("wt", [KP, KT, cA], dt.bfloat16, kind="ExternalInput")
    wa = nc.dram_tensor("wa", [KP, KT, cB], dt.bfloat16, kind="ExternalInput")
    g1i = nc.dram_tensor("g1i", [P, sched.idxcols], dt.int16, kind="ExternalInput")
    ohd = nc.dram_tensor("ohd", [P, sched.TT, 2, P], oh_mdt, kind="ExternalInput")
    avec = nc.dram_tensor("avec", [P, 1], dt.float32, kind="ExternalInput")
    out = nc.dram_tensor("out", [cfg.NW * P, HID], dt.float32, kind="ExternalOutput")

    with TileContext(nc) as tc:
        with tc.tile_pool(name="const", bufs=1) as cpool, \
             tc.tile_pool(name="dram", bufs=1, space="DRAM") as dpool:
            table = dpool.tile([cfg.NPAD, cfg.row_bf], dt.bfloat16)
            wt_sb = cpool.tile([KP, KT, cA], dt.bfloat16)
            nc.sync.dma_start(out=wt_sb[:], in_=wt[:, :, :])
            wa_sb = cpool.tile([KP, KT, cB], dt.bfloat16)
            nc.sync.dma_start(out=wa_sb[:], in_=wa[:, :, :])
            if phases == "full":
                a_sb = cpool.tile([P, 1], dt.float32)
                nc.sync.dma_start(out=a_sb[:], in_=avec[:, :])
            if phases in ("full", "p1g"):
                g1i_sb = cpool.tile([P, sched.idxcols], dt.int16)
                nc.sync.dma_start(out=g1i_sb[:], in_=g1i[:, :])

            # ---------------- phase 1: projection table ----------------
            # Two balanced column streams (colsA=256 incl. the 16 score cols,
            # colsB=272) with interleaved matmuls so LDWEIGHTS always hides
            # under the previous stream.  Rows are written 1088B of 1280B
            # (the 192B tail is never read).
            used_bf = cfg.row_used // 2              # 544 bf16 per row
            with tc.tile_pool(name="p1", bufs=3) as xpool, \
                 tc.tile_pool(name="p1ps", bufs=2, space="PSUM") as pspool, \
                 tc.tile_pool(name="p1st", bufs=3) as stpool:
                for j in range(cfg.NT):
                    xtile = xpool.tile([KP, KT, P], dt.bfloat16, tag="x")
                    nc.sync.dma_start(out=xtile[:], in_=xt[:, j, :, :])
                    # full-bank tiles: a matmul output must stay inside one
                    # 2KB PSUM bank, and concurrent accumulation groups must
                    # live in different banks.
                    psA_f = pspool.tile([P, 512], dt.float32, tag="psA")
                    psB_f = pspool.tile([P, 512], dt.float32, tag="psB")
                    psA = psA_f[:, 0:cA]
                    psB = psB_f[:, 0:cB]
                    if cfg.p1_split:
                        for ki in range(KT):
                            nc.tensor.matmul(psA[:], xtile[:, ki, :], wt_sb[:, ki, :],
                                             start=(ki == 0), stop=(ki == KT - 1))
                            nc.tensor.matmul(psB[:], xtile[:, ki, :], wa_sb[:, ki, :],
                                             start=(ki == 0), stop=(ki == KT - 1))
                    else:
                        for ki in range(KT):
                            nc.tensor.matmul(psA[:], xtile[:, ki, :], wt_sb[:, ki, :],
                                             start=(ki == 0), stop=(ki == KT - 1))
                        for ki in range(KT):
                            nc.tensor.matmul(psB[:], xtile[:, ki, :], wa_sb[:, ki, :],
                                             start=(ki == 0), stop=(ki == KT - 1))
                    stg = stpool.tile([P, used_bf], dt.bfloat16, tag="stg")
                    stg32 = stg.bitcast(dt.float32)
                    if cfg.p1_split:
                        nc.scalar.copy(out=stg[:, 0:cfg.splitA],
                                       in_=psA[:, 0:cfg.splitA])
                        nc.scalar.copy(
                            out=stg32[:, cfg.s_src_off:cfg.s_src_off + 2 * H],
                            in_=psA[:, cfg.splitA:cfg.colsA])
                        nc.vector.tensor_copy(out=stg[:, cfg.splitA:HID], in_=psB[:])
                    else:
                        nc.scalar.copy(out=stg[:, 0:HID], in_=psA[:])
                        nc.scalar.copy(
                            out=stg32[:, cfg.s_src_off:cfg.s_src_off + 2 * H],
                            in_=psB[:])
                    nc.sync.dma_start(
                        out=table[j * P:(j + 1) * P, 0:used_bf], in_=stg[:])

            tc.strict_bb_all_engine_barrier()

            # ---------------- phase 1.5: resident s_tgt (hi/lo bf16) ----------------
            pid = nc.sync.partition_id()
            table32 = table.bitcast(dt.float32)
            s_ap = table32[bass.DynSlice(pid * cfg.shard, cfg.NW * P),
                           cfg.s_tgt_off:cfg.s_tgt_off + H]
            s_ap = s_ap.rearrange("(w p) h -> p w h", p=P)
            s_all = cpool.tile([P, cfg.NW, H], dt.float32)
            nc.sync.dma_start(out=s_all[:], in_=s_ap)
            s_hilo = cpool.tile([P, cfg.NW, 2, H], dt.bfloat16)
            s_hi32 = cpool.tile([P, cfg.NW, H], dt.float32)
            nc.vector.tensor_copy(out=s_hilo[:, :, 0, :], in_=s_all[:])
            nc.vector.tensor_copy(out=s_hi32[:], in_=s_hilo[:, :, 0, :])
            nc.vector.tensor_tensor(out=s_hilo[:, :, 1, :], in0=s_all[:],
                                    in1=s_hi32[:], op=mybir.AluOpType.subtract)

            # ---------------- phase 2: window groups ----------------
            CH = cfg.CH
            with tc.tile_pool(name="p2", bufs=2) as pool, \
                 tc.tile_pool(name="p2c", bufs=2) as cpool2, \
                 tc.tile_pool(name="p2ps", bufs=2, space="PSUM") as pps, \
                 tc.tile_pool(name="p2acc", bufs=1, space="PSUM") as apool:
                # Zero both g1t rotation buffers once: slots whose gather was
                # trimmed (trailing -1 indices) read stale SBUF, which must be
                # finite.  After the first two groups, stale bytes are old
                # gathered rows (finite bf16/f32), so one round suffices.
                for _ in range(2):
                    g1z = pool.tile([P, sched.TGmax, cfg.row_bf], dt.bfloat16,
                                    tag="g1t")
                    nc.vector.memset(g1z[:], 0.0)
                for g, grp in enumerate(sched.groups):
                    Tg = sched.TG[g]
                    nw = len(grp)
                    g1t = pool.tile([P, sched.TGmax, cfg.row_bf], dt.bfloat16,
                                    tag="g1t")
                    for (b, slot_off, nslots, idxcol0) in sched.calls[g]:
                        rows = min(cfg.NPAD, (b + 1) * cfg.bucket) - b * cfg.bucket
                        # single_packet chains the call's descriptors into
                        # one SDMA packet; the HW packet limit is 64
                        # descriptors, so large merged calls must split.
                        nc.gpsimd.dma_gather(
                            g1t[:, slot_off // P:(slot_off + nslots) // P, :],
                            table[b * cfg.bucket:b * cfg.bucket + rows, :],
                            g1i_sb[:, idxcol0:idxcol0 + nslots // 16],
                            nslots, nslots, cfg.row_bf,
                            single_packet=(nslots // 16 + 1 <= 64))
                    g1t32 = g1t.bitcast(dt.float32)
                    jb = sched.tile_base[g]
                    agg = apool.tile([P, cfg.GW, HID], dt.float32, tag="agg")
                    den = apool.tile([P, cfg.GW, H], dt.float32, tag="den")
                    den_acc = pool.tile([P, cfg.GW, H], dt.float32, tag="den_acc")
                    den_seen = set()
                    started = [False] * nw
                    for c in range(_ceil(Tg, CH)):
                        t0, t1 = c * CH, min(Tg, (c + 1) * CH)
                        tn = t1 - t0
                        ohc = cpool2.tile([P, CH, 2, P], oh_mdt, tag="ohc")
                        nc.sync.dma_start(out=ohc[:, :tn, :, :],
                                          in_=ohd[:, jb + t0:jb + t1, :, :])
                        stgt = pps.tile([P, CH, 2, H], dt.float32, tag="stgt")
                        for t in range(t0, t1):
                            nc.tensor.matmul(
                                stgt[:, t - t0, :, :], ohc[:, t - t0, 1, :],
                                s_hilo[:, grp[sched.tile_w[g][t]], :, :],
                                start=True, stop=True)
                        s_sum = cpool2.tile([P, CH, H], dt.float32, tag="s_sum")
                        s_act = cpool2.tile([P, CH, H], dt.float32, tag="s_act")
                        nc.vector.tensor_tensor(
                            out=s_sum[:, :tn, :], in0=stgt[:, :tn, 0, :],
                            in1=g1t32[:, t0:t1, cfg.s_src_off:cfg.s_src_off + H],
                            op=mybir.AluOpType.add)
                        nc.vector.tensor_tensor(
                            out=s_act[:, :tn, :], in0=stgt[:, :tn, 1, :],
                            in1=s_sum[:, :tn, :], op=mybir.AluOpType.add)
                        nc.vector.scalar_tensor_tensor(
                            out=s_sum[:, :tn, :], in0=s_act[:, :tn, :],
                            scalar=cfg.leak, in1=s_act[:, :tn, :],
                            op0=mybir.AluOpType.mult, op1=mybir.AluOpType.max)
                        exp_t = cpool2.tile([P, CH, H], dt.bfloat16, tag="exp_t")
                        nc.scalar.activation(out=exp_t[:, :tn, :],
                                             in_=s_sum[:, :tn, :],
                                             func=mybir.ActivationFunctionType.Exp)

                        w_t = cpool2.tile([P, CH, HID], dt.bfloat16, tag="w_t")
                        proj4 = g1t[:, t0:t1, 0:HID].rearrange(
                            "p t (h f) -> p t h f", h=H)
                        exp4 = _bcast_last(exp_t[:, :tn, :], cfg.F)
                        out4 = w_t[:, :tn, :].rearrange("p t (h f) -> p t h f", h=H)
                        nc.vector.tensor_tensor(out=out4, in0=proj4, in1=exp4,
                                                op=mybir.AluOpType.mult)

                        # tiles grouped by window: agg chains span the whole
                        # group (each window's agg is its own PSUM bank); den
                        # groups open/close within this chunk (all windows
                        # share one bank, so groups must not interleave), and
                        # chunk partials accumulate into den_acc on DVE.
                        bywin = {}
                        for t in range(t0, t1):
                            bywin.setdefault(sched.tile_w[g][t], []).append(t)
                        for wi, tlist in bywin.items():
                            for t in tlist:
                                first = not started[wi]
                                started[wi] = True
                                lastt = (t == sched.win_last_tile[g][wi])
                                nc.tensor.matmul(agg[:, wi, :],
                                                 ohc[:, t - t0, 0, :],
                                                 w_t[:, t - t0, :],
                                                 start=first, stop=lastt)
                                nc.tensor.matmul(den[:, wi, :],
                                                 ohc[:, t - t0, 0, :],
                                                 exp_t[:, t - t0, :],
                                                 start=(t == tlist[0]),
                                                 stop=(t == tlist[-1]))
                            if wi in den_seen:
                                nc.vector.tensor_tensor(
                                    out=den_acc[:, wi, :], in0=den[:, wi, :],
                                    in1=den_acc[:, wi, :], op=mybir.AluOpType.add)
                            else:
                                nc.vector.tensor_copy(out=den_acc[:, wi, :],
                                                      in_=den[:, wi, :])
                                den_seen.add(wi)

                    # flush: softmax divide + PReLU for all windows in group
                    den_sb = pool.tile([P, cfg.GW, H], dt.float32, tag="den_sb")
                    recip = pool.tile([P, cfg.GW, H], dt.float32, tag="recip")
                    nc.vector.tensor_scalar_add(out=den_sb[:, :nw, :],
                                                in0=den_acc[:, :nw, :],
                                                scalar1=1e-16)
                    nc.vector.reciprocal(out=recip[:, :nw, :], in_=den_sb[:, :nw, :])
                    for wi, w in enumerate(grp):
                        z = pool.tile([P, HID], dt.float32, tag="z")
                        agg4 = agg[:, wi, :].rearrange("p (h f) -> p h f", h=H)
                        z4 = z[:].rearrange("p (h f) -> p h f", h=H)
                        nc.vector.tensor_tensor(
                            out=z4, in0=agg4,
                            in1=_bcast_last(recip[:, wi, :], cfg.F),
                            op=mybir.AluOpType.mult)
                        res = pool.tile([P, HID], dt.float32, tag="res")
                        nc.vector.scalar_tensor_tensor(
                            out=res[:], in0=z[:], scalar=a_sb[:, 0:1], in1=z[:],
                            op0=mybir.AluOpType.mult, op1=mybir.AluOpType.max)
                        nc.sync.dma_start(out=out[w * P:(w + 1) * P, :], in_=res[:])

    nc.compile()
    return nc


def prepare(cfg: Cfg, inputs):
    """Host-side prep shared by HW and sim paths.

    Returns (sched, in_maps, assemble) where assemble(core_outs) -> full out.
    """
    X = np.asarray(inputs["in_nodes_features"], np.float32)
    ei = np.asarray(inputs["edge_index"], np.int64)
    W = np.asarray(inputs["W"], np.float32)
    b_lin = np.asarray(inputs["b_lin"], np.float32)
    a_src = np.asarray(inputs["a_src"], np.float32)
    a_tgt = np.asarray(inputs["a_tgt"], np.float32)
    bias = np.asarray(inputs["bias"], np.float32)
    prelu_a = float(np.asarray(inputs["prelu_a"], np.float32))

    assert np.all(b_lin == 0) and np.all(bias == 0), "nonzero bias unsupported"
    assert 0.0 <= prelu_a <= 1.0, "prelu_a outside [0,1] unsupported"

    src, trg = ei[0], ei[1]
    core_of = trg // cfg.shard
    win_of = (trg % cfg.shard) // P
    buck_of = src // cfg.bucket
    counts = np.zeros((cfg.ncores, cfg.NW, cfg.NB), np.int64)
    for k in range(cfg.ncores):
        m = core_of == k
        counts[k] = np.bincount(
            win_of[m] * cfg.NB + buck_of[m],
            minlength=cfg.NW * cfg.NB).reshape(cfg.NW, cfg.NB)
    sched = build_schedule(cfg, counts)

    xt = pack_xt(cfg, X)
    wtp, wap = pack_w(cfg, W, a_src, a_tgt)
    av = np.full((P, 1), prelu_a, np.float32)

    in_maps = []
    for k in range(cfg.ncores):
        g1i_k, oh_k = prep_core(cfg, sched, src, trg, k)
        in_maps.append({
            "xt": xt, "wt": wtp, "wa": wap,
            "g1i": g1i_k, "ohd": oh_k, "avec": av,
        })

    def assemble(core_outs):
        return np.concatenate(
            [np.asarray(o["out"][: cfg.shard], np.float32) for o in core_outs], axis=0)

    return sched, in_maps, assemble


_BUILT = {}


def _get_built(cfg: Cfg, sched: Schedule):
    key = (cfg.N, cfg.E, cfg.HID, cfg.HEADS, cfg.ncores, cfg.bucket,
           tuple(sched.TW), sched.idxcols)
    if key not in _BUILT:
        _BUILT[key] = build_nc(cfg, sched)
    return _BUILT[key]


def kernel(**inputs):
    from concourse.bass_utils import run_bass_kernel_spmd

    cfg = Cfg()
    sched, in_maps, assemble = prepare(cfg, inputs)
    nc = _get_built(cfg, sched)
    res = run_bass_kernel_spmd(nc, in_maps, core_ids=list(range(cfg.ncores)))
    return assemble(res.results)



# revision 22
# speedup vs baseline: 1.2519x; 1.2347x over previous
"""GAT layer (multi-head graph attention) on 8 TRN2 NeuronCores.

Strategy (per sharding hint): destination nodes are sharded across the 8
cores.  Each core:
  phase 1: computes the full projection table redundantly (bf16 GEMM
           X @ W.T plus the per-head attention score reductions), packed
           as [proj bf16 | s_src f32 | s_tgt f32 | pad] rows in local HBM.
  phase 2: walks its shard's destination windows (128 targets / window).
           Edges are pre-sorted by (window, src-bucket) on the host;
           dma_gather pulls the source rows (int16 indices per 32768-row
           bucket), scores -> leaky-relu -> exp run batched per window,
           and one-hot matmuls (host-streamed) accumulate both the
           softmax denominator and the weighted aggregation in PSUM.
           Softmax division + PReLU happen once per window at flush.

kernel(**inputs) takes the FULL inputs and returns the FULL output.
"""

import math
from dataclasses import dataclass, field

import numpy as np
import ml_dtypes

BF16 = ml_dtypes.bfloat16
P = 128


def _ceil(a, b):
    return -(-a // b)


@dataclass
class Cfg:
    N: int = 100000
    E: int = 800000
    HID: int = 512
    HEADS: int = 8
    ncores: int = 8
    bucket: int = 32768
    leak: float = 0.01
    oh_bf16: bool = True  # one-hot stream dtype (bf16; fp8 is an option)
    GW: int = 1           # windows per gather group
    CH: int = 16          # tiles per phase-2 compute chunk
    p1_split: bool = True  # split-GEMM phase 1 (interleaved psA/psB)

    def __post_init__(self):
        assert self.N % self.ncores == 0
        assert self.bucket <= 32768
        self.F = self.HID // self.HEADS
        self.shard = self.N // self.ncores
        self.NW = _ceil(self.shard, P)          # windows per core
        self.NB = _ceil(self.N, self.bucket)    # src buckets (int16 range)
        self.NT = _ceil(self.N, P)              # projection tiles
        self.NPAD = self.NT * P
        self.KP = min(self.HID, P)              # contraction partitions
        self.KT = self.HID // self.KP           # contraction tiles
        row_bytes = self.HID * 2 + 2 * self.HEADS * 4
        self.row_used = row_bytes               # bytes actually written
        self.row_bytes = _ceil(row_bytes, 256) * 256
        self.row_bf = self.row_bytes // 2
        self.row_f32 = self.row_bytes // 4
        self.s_src_off = self.HID // 2          # f32 col of s_src in a row
        self.s_tgt_off = self.HID // 2 + self.HEADS
        # phase-1 output split: colsA covers proj[0:splitA] plus the 2H score
        # columns; colsB covers proj[splitA:HID].  Streams are balanced so
        # every LDWEIGHTS hides under the previous matmul's column stream.
        self.splitA = self.HID // 2 - self.HEADS * 2  # 240
        self.colsA = self.splitA + 2 * self.HEADS     # 256
        self.colsB = self.HID - self.splitA           # 272


@dataclass
class Schedule:
    """Core-independent (uniform) phase-2 schedule.

    Windows are processed in groups of GW; each gather call covers one
    (group, bucket) pair so the ~4us fixed per-call GpSimd cost is paid
    ~NW/GW*NB times instead of NW*NB times.  Within a group slots are laid
    out bucket-major: [b0: w0|w1|w2|w3, b1: w0|..., ...], each (w,b)
    segment padded to a 128 multiple so no tile mixes windows.
    """
    seg: np.ndarray          # [NW, NB] slot counts (128-aligned, global max)
    groups: list             # list of lists of window indices
    TG: list                 # tiles per group
    TGmax: int
    calls: list              # per group: list of (b, slot_off, nslots, idxcol0)
    seg_off: dict            # (w, b) -> slot offset within its group
    tile_w: list             # per group: window index (in-group) per tile
    win_last_tile: list      # per group: in-group last tile idx per window
    idxcols: int             # total int16 idx columns (per 16-wrap row)
    TT: int                  # total tiles
    tile_base: list          # first global tile index of each group


def build_schedule(cfg: Cfg, counts: np.ndarray) -> Schedule:
    """counts: [ncores, NW, NB] edge counts."""
    maxcnt = counts.max(axis=0)  # [NW, NB]
    seg = np.where(maxcnt > 0, _ceil(maxcnt, P) * P, 0).astype(np.int64)
    groups = [list(range(g0, min(g0 + cfg.GW, cfg.NW)))
              for g0 in range(0, cfg.NW, cfg.GW)]
    TG, calls, tile_base, tile_w, win_last_tile = [], [], [], [], []
    seg_off = {}
    idxcol = 0
    tt = 0
    for grp in groups:
        tile_base.append(tt)
        gcalls = []
        tw = []
        last = {wi: -1 for wi in range(len(grp))}
        off = 0
        for b in range(cfg.NB):
            nslots = int(sum(seg[w, b] for w in grp))
            if nslots == 0:
                continue
            gcalls.append((b, off, nslots, idxcol))
            for wi, w in enumerate(grp):
                s = int(seg[w, b])
                if s == 0:
                    continue
                seg_off[(w, b)] = off
                for _ in range(s // P):
                    last[wi] = len(tw)
                    tw.append(wi)
                off += s
            idxcol += nslots // 16
        assert off % P == 0
        TG.append(off // P)
        tt += off // P
        calls.append(gcalls)
        tile_w.append(tw)
        win_last_tile.append(last)
    return Schedule(seg=seg, groups=groups, TG=TG, TGmax=max(TG), calls=calls,
                    seg_off=seg_off, tile_w=tile_w, win_last_tile=win_last_tile,
                    idxcols=idxcol, TT=tt, tile_base=tile_base)


def prep_core(cfg: Cfg, sched: Schedule, src, trg, k):
    """Per-core input arrays: g1 idx stream and one-hot stream.

    Padding slots get index 0 (the bucket's first row: real, finite data)
    and all-zero one-hot columns, so they contribute nothing to the
    aggregation or the softmax denominator.
    """
    oh_dt = BF16 if cfg.oh_bf16 else ml_dtypes.float8_e4m3
    mask = (trg // cfg.shard) == k
    esrc = src[mask]
    etrg = trg[mask]
    trel = etrg - k * cfg.shard
    win = trel // P
    buck = esrc // cfg.bucket
    # order edges by (window, bucket); stable so host/device agree
    order = np.lexsort((buck, win))
    esrc, etrg, trel, win, buck = (a[order] for a in (esrc, etrg, trel, win, buck))

    g1i = np.zeros((P, sched.idxcols), np.int16)
    oh = np.zeros((P, sched.TT, 2, P), oh_dt)

    # per (window, bucket) segment boundaries
    key = win * cfg.NB + buck
    starts = np.searchsorted(key, np.arange(cfg.NW * cfg.NB), side="left")
    ends = np.searchsorted(key, np.arange(cfg.NW * cfg.NB), side="right")

    for g, grp in enumerate(sched.groups):
        for (b, call_off, nslots, idxcol0) in sched.calls[g]:
            idx = np.zeros(nslots, np.int16)
            pos = 0
            pad_last = 0
            for w in grp:
                s = int(sched.seg[w, b])
                if s == 0:
                    continue
                lo, hi = int(starts[w * cfg.NB + b]), int(ends[w * cfg.NB + b])
                cnt = hi - lo
                assert cnt <= s
                idx[pos:pos + cnt] = (esrc[lo:hi] - b * cfg.bucket).astype(np.int16)
                # idx[pos+cnt : pos+s] stays 0 (mid-call padding)
                pad_last = s - cnt
                # one-hots for this segment's slots
                tloc = (trel[lo:hi] - w * P).astype(np.int64)   # [cnt] in [0,128)
                gslot = call_off + pos + np.arange(cnt)
                tgl = sched.tile_base[g] + gslot // P
                oh[gslot % P, tgl, 0, tloc] = oh_dt(1.0)
                oh[tloc, tgl, 1, gslot % P] = oh_dt(1.0)
                pos += s
            assert pos == nslots
            # NOTE: trailing -1 indices (ucode-side trim) hang this deployment
            # -- all padding stays index 0 (gathers the bucket's first row).
            del pad_last
            blk = idx.reshape(nslots // 16, 16).T          # [16, cols]
            g1i[:, idxcol0:idxcol0 + nslots // 16] = np.tile(blk, (8, 1))
    return g1i, oh


def pack_xt(cfg: Cfg, X: np.ndarray) -> np.ndarray:
    """X [N, HID] f32 -> bf16 packed [KP, NT, KT, P]: (p, j, ki, n) = X[j*P+n, ki*KP+p]."""
    Xp = np.zeros((cfg.NPAD, cfg.HID), np.float32)
    Xp[: cfg.N] = X
    Xb = Xp.astype(BF16)
    # [NT, P(n), KT, KP(p)] -> transpose to [KP, NT, KT, P]
    v = Xb.reshape(cfg.NT, P, cfg.KT, cfg.KP)
    return np.ascontiguousarray(v.transpose(3, 0, 2, 1))


def pack_w(cfg: Cfg, W, a_src, a_tgt):
    """Returns wA [KP, KT, colsA] and wB [KP, KT, colsB] bf16.

    wA = [W.T[:, :splitA] | WA] (proj prefix plus both score projections),
    wB = W.T[:, splitA:].  Splitting the 528 output columns into two
    balanced streams lets every LDWEIGHTS hide under a matmul stream.
    """
    WT = W.T.astype(np.float32)                       # [HID(d), HID(o)]
    wa_s = (W.reshape(cfg.HEADS, cfg.F, cfg.HID)
            * np.asarray(a_src, np.float32).reshape(cfg.HEADS, cfg.F, 1)).sum(1)  # [H, d]
    wa_t = (W.reshape(cfg.HEADS, cfg.F, cfg.HID)
            * np.asarray(a_tgt, np.float32).reshape(cfg.HEADS, cfg.F, 1)).sum(1)
    WA = np.concatenate([wa_s.T, wa_t.T], axis=1)     # [d, 2H]
    if not cfg.p1_split:
        wAf, wBf = WT, WA      # original layout: full W.T stream + score stream
    else:
        wAf = np.concatenate([WT[:, :cfg.splitA], WA], axis=1)   # [d, colsA]
        wBf = WT[:, cfg.splitA:]                                  # [d, colsB]
    wA = np.ascontiguousarray(
        wAf.astype(BF16).reshape(cfg.KT, cfg.KP, wAf.shape[1]).transpose(1, 0, 2))
    wB = np.ascontiguousarray(
        wBf.astype(BF16).reshape(cfg.KT, cfg.KP, wBf.shape[1]).transpose(1, 0, 2))
    return wA, wB


def _bcast_last(ap, n):
    """Append a 0-stride broadcast dim of size n to an AP."""
    import concourse.bass as bass
    lst = [list(x) for x in ap.ap] + [[0, n]]
    return bass.AP(ap.tensor, ap.offset, lst)


def build_nc(cfg: Cfg, sched: Schedule, phases: str = "full"):
    import concourse.bacc as bacc
    import concourse.bass as bass
    import concourse.mybir as mybir
    from concourse.tile import TileContext

    dt = mybir.dt
    oh_mdt = dt.bfloat16 if cfg.oh_bf16 else dt.float8e4
    H, HID, KT, KP = cfg.HEADS, cfg.HID, cfg.KT, cfg.KP

    nc = bacc.Bacc("TRN2", target_bir_lowering=False)

    cA = cfg.colsA if cfg.p1_split else HID
    cB = cfg.colsB if cfg.p1_split else 2 * H
    xt = nc.dram_tensor("xt", [KP, cfg.NT, KT, P], dt.bfloat16, kind="ExternalInput")
    wt = nc.dram_tensor("wt", [KP, KT, cA], dt.bfloat16, kind="ExternalInput")
    wa = nc.dram_tensor("wa", [KP, KT, cB], dt.bfloat16, kind="ExternalInput")
    g1i = nc.dram_tensor("g1i", [P, sched.idxcols], dt.int16, kind="ExternalInput")
    ohd = nc.dram_tensor("ohd", [P, sched.TT, 2, P], oh_mdt, kind="ExternalInput")
    avec = nc.dram_tensor("avec", [P, 1], dt.float32, kind="ExternalInput")
    out = nc.dram_tensor("out", [cfg.NW * P, HID], dt.float32, kind="ExternalOutput")

    with TileContext(nc) as tc:
        with tc.tile_pool(name="const", bufs=1) as cpool, \
             tc.tile_pool(name="dram", bufs=1, space="DRAM") as dpool:
            table = dpool.tile([cfg.NPAD, cfg.row_bf], dt.bfloat16)
            wt_sb = cpool.tile([KP, KT, cA], dt.bfloat16)
            nc.sync.dma_start(out=wt_sb[:], in_=wt[:, :, :])
            wa_sb = cpool.tile([KP, KT, cB], dt.bfloat16)
            nc.sync.dma_start(out=wa_sb[:], in_=wa[:, :, :])
            if phases == "full":
                a_sb = cpool.tile([P, 1], dt.float32)
                nc.sync.dma_start(out=a_sb[:], in_=avec[:, :])
            if phases in ("full", "p1g"):
                g1i_sb = cpool.tile([P, sched.idxcols], dt.int16)
                nc.sync.dma_start(out=g1i_sb[:], in_=g1i[:, :])

            # ---------------- phase 1: projection table ----------------
            # Two balanced column streams (colsA=256 incl. the 16 score cols,
            # colsB=272) with interleaved matmuls so LDWEIGHTS always hides
            # under the previous stream.  Rows are written 1088B of 1280B
            # (the 192B tail is never read).
            used_bf = cfg.row_used // 2              # 544 bf16 per row
            with tc.tile_pool(name="p1", bufs=3) as xpool, \
                 tc.tile_pool(name="p1ps", bufs=2, space="PSUM") as pspool, \
                 tc.tile_pool(name="p1st", bufs=3) as stpool:
                for j in range(cfg.NT):
                    xtile = xpool.tile([KP, KT, P], dt.bfloat16, tag="x")
                    nc.sync.dma_start(out=xtile[:], in_=xt[:, j, :, :])
                    # full-bank tiles: a matmul output must stay inside one
                    # 2KB PSUM bank, and concurrent accumulation groups must
                    # live in different banks.
                    psA_f = pspool.tile([P, 512], dt.float32, tag="psA")
                    psB_f = pspool.tile([P, 512], dt.float32, tag="psB")
                    psA = psA_f[:, 0:cA]
                    psB = psB_f[:, 0:cB]
                    if cfg.p1_split:
                        for ki in range(KT):
                            nc.tensor.matmul(psA[:], xtile[:, ki, :], wt_sb[:, ki, :],
                                             start=(ki == 0), stop=(ki == KT - 1))
                            nc.tensor.matmul(psB[:], xtile[:, ki, :], wa_sb[:, ki, :],
                                             start=(ki == 0), stop=(ki == KT - 1))
                    else:
                        for ki in range(KT):
                            nc.tensor.matmul(psA[:], xtile[:, ki, :], wt_sb[:, ki, :],
                                             start=(ki == 0), stop=(ki == KT - 1))
                        for ki in range(KT):
                            nc.tensor.matmul(psB[:], xtile[:, ki, :], wa_sb[:, ki, :],
                                             start=(ki == 0), stop=(ki == KT - 1))
                    stg = stpool.tile([P, used_bf], dt.bfloat16, tag="stg")
                    stg32 = stg.bitcast(dt.float32)
                    if cfg.p1_split:
                        nc.scalar.copy(out=stg[:, 0:cfg.splitA],
                                       in_=psA[:, 0:cfg.splitA])
                        nc.scalar.copy(
                            out=stg32[:, cfg.s_src_off:cfg.s_src_off + 2 * H],
                            in_=psA[:, cfg.splitA:cfg.colsA])
                        nc.vector.tensor_copy(out=stg[:, cfg.splitA:HID], in_=psB[:])
                    else:
                        nc.scalar.copy(out=stg[:, 0:HID], in_=psA[:])
                        nc.scalar.copy(
                            out=stg32[:, cfg.s_src_off:cfg.s_src_off + 2 * H],
                            in_=psB[:])
                    nc.sync.dma_start(
                        out=table[j * P:(j + 1) * P, 0:used_bf], in_=stg[:])

            tc.strict_bb_all_engine_barrier()

            # ---------------- phase 1.5: resident s_tgt (hi/lo bf16) ----------------
            pid = nc.sync.partition_id()
            table32 = table.bitcast(dt.float32)
            s_ap = table32[bass.DynSlice(pid * cfg.shard, cfg.NW * P),
                           cfg.s_tgt_off:cfg.s_tgt_off + H]
            s_ap = s_ap.rearrange("(w p) h -> p w h", p=P)
            s_all = cpool.tile([P, cfg.NW, H], dt.float32)
            nc.sync.dma_start(out=s_all[:], in_=s_ap)
            s_hilo = cpool.tile([P, cfg.NW, 2, H], dt.bfloat16)
            s_hi32 = cpool.tile([P, cfg.NW, H], dt.float32)
            nc.vector.tensor_copy(out=s_hilo[:, :, 0, :], in_=s_all[:])
            nc.vector.tensor_copy(out=s_hi32[:], in_=s_hilo[:, :, 0, :])
            nc.vector.tensor_tensor(out=s_hilo[:, :, 1, :], in0=s_all[:],
                                    in1=s_hi32[:], op=mybir.AluOpType.subtract)

            # ---------------- phase 2: window groups ----------------
            CH = cfg.CH
            with tc.tile_pool(name="p2", bufs=2) as pool, \
                 tc.tile_pool(name="p2c", bufs=2) as cpool2, \
                 tc.tile_pool(name="p2ps", bufs=2, space="PSUM") as pps, \
                 tc.tile_pool(name="p2acc", bufs=1, space="PSUM") as apool:
                # Zero both g1t rotation buffers once: slots whose gather was
                # trimmed (trailing -1 indices) read stale SBUF, which must be
                # finite.  After the first two groups, stale bytes are old
                # gathered rows (finite bf16/f32), so one round suffices.
                for _ in range(2):
                    g1z = pool.tile([P, sched.TGmax, cfg.row_bf], dt.bfloat16,
                                    tag="g1t")
                    nc.vector.memset(g1z[:], 0.0)
                for g, grp in enumerate(sched.groups):
                    Tg = sched.TG[g]
                    nw = len(grp)
                    g1t = pool.tile([P, sched.TGmax, cfg.row_bf], dt.bfloat16,
                                    tag="g1t")
                    for (b, slot_off, nslots, idxcol0) in sched.calls[g]:
                        rows = min(cfg.NPAD, (b + 1) * cfg.bucket) - b * cfg.bucket
                        # single_packet chains the call's descriptors into
                        # one SDMA packet; the HW packet limit is 64
                        # descriptors, so large merged calls must split.
                        nc.gpsimd.dma_gather(
                            g1t[:, slot_off // P:(slot_off + nslots) // P, :],
                            table[b * cfg.bucket:b * cfg.bucket + rows, :],
                            g1i_sb[:, idxcol0:idxcol0 + nslots // 16],
                            nslots, nslots, cfg.row_bf,
                            single_packet=(nslots // 16 + 1 <= 64))
                    g1t32 = g1t.bitcast(dt.float32)
                    jb = sched.tile_base[g]
                    agg = apool.tile([P, cfg.GW, HID], dt.float32, tag="agg")
                    den = apool.tile([P, cfg.GW, H], dt.float32, tag="den")
                    den_acc = pool.tile([P, cfg.GW, H], dt.float32, tag="den_acc")
                    den_seen = set()
                    started = [False] * nw
                    for c in range(_ceil(Tg, CH)):
                        t0, t1 = c * CH, min(Tg, (c + 1) * CH)
                        tn = t1 - t0
                        ohc = cpool2.tile([P, CH, 2, P], oh_mdt, tag="ohc")
                        nc.sync.dma_start(out=ohc[:, :tn, :, :],
                                          in_=ohd[:, jb + t0:jb + t1, :, :])
                        stgt = pps.tile([P, CH, 2, H], dt.float32, tag="stgt")
                        for t in range(t0, t1):
                            nc.tensor.matmul(
                                stgt[:, t - t0, :, :], ohc[:, t - t0, 1, :],
                                s_hilo[:, grp[sched.tile_w[g][t]], :, :],
                                start=True, stop=True)
                        s_sum = cpool2.tile([P, CH, H], dt.float32, tag="s_sum")
                        s_act = cpool2.tile([P, CH, H], dt.float32, tag="s_act")
                        nc.vector.tensor_tensor(
                            out=s_sum[:, :tn, :], in0=stgt[:, :tn, 0, :],
                            in1=g1t32[:, t0:t1, cfg.s_src_off:cfg.s_src_off + H],
                            op=mybir.AluOpType.add)
                        nc.vector.tensor_tensor(
                            out=s_act[:, :tn, :], in0=stgt[:, :tn, 1, :],
                            in1=s_sum[:, :tn, :], op=mybir.AluOpType.add)
                        nc.vector.scalar_tensor_tensor(
                            out=s_sum[:, :tn, :], in0=s_act[:, :tn, :],
                            scalar=cfg.leak, in1=s_act[:, :tn, :],
                            op0=mybir.AluOpType.mult, op1=mybir.AluOpType.max)
                        exp_t = cpool2.tile([P, CH, H], dt.bfloat16, tag="exp_t")
                        nc.scalar.activation(out=exp_t[:, :tn, :],
                                             in_=s_sum[:, :tn, :],
                                             func=mybir.ActivationFunctionType.Exp)

                        w_t = cpool2.tile([P, CH, HID], dt.bfloat16, tag="w_t")
                        proj4 = g1t[:, t0:t1, 0:HID].rearrange(
                            "p t (h f) -> p t h f", h=H)
                        exp4 = _bcast_last(exp_t[:, :tn, :], cfg.F)
                        out4 = w_t[:, :tn, :].rearrange("p t (h f) -> p t h f", h=H)
                        nc.vector.tensor_tensor(out=out4, in0=proj4, in1=exp4,
                                                op=mybir.AluOpType.mult)

                        # tiles grouped by window: agg chains span the whole
                        # group (each window's agg is its own PSUM bank); den
                        # groups open/close within this chunk (all windows
                        # share one bank, so groups must not interleave), and
                        # chunk partials accumulate into den_acc on DVE.
                        bywin = {}
                        for t in range(t0, t1):
                            bywin.setdefault(sched.tile_w[g][t], []).append(t)
                        for wi, tlist in bywin.items():
                            for t in tlist:
                                first = not started[wi]
                                started[wi] = True
                                lastt = (t == sched.win_last_tile[g][wi])
                                nc.tensor.matmul(agg[:, wi, :],
                                                 ohc[:, t - t0, 0, :],
                                                 w_t[:, t - t0, :],
                                                 start=first, stop=lastt)
                                nc.tensor.matmul(den[:, wi, :],
                                                 ohc[:, t - t0, 0, :],
                                                 exp_t[:, t - t0, :],
                                                 start=(t == tlist[0]),
                                                 stop=(t == tlist[-1]))
                            if wi in den_seen:
                                nc.vector.tensor_tensor(
                                    out=den_acc[:, wi, :], in0=den[:, wi, :],
                                    in1=den_acc[:, wi, :], op=mybir.AluOpType.add)
                            else:
                                nc.vector.tensor_copy(out=den_acc[:, wi, :],
                                                      in_=den[:, wi, :])
                                den_seen.add(wi)

                    # flush: softmax divide + PReLU for all windows in group
                    den_sb = pool.tile([P, cfg.GW, H], dt.float32, tag="den_sb")
                    recip = pool.tile([P, cfg.GW, H], dt.float32, tag="recip")
                    nc.vector.tensor_scalar_add(out=den_sb[:, :nw, :],
                                                in0=den_acc[:, :nw, :],
                                                scalar1=1e-16)
                    nc.vector.reciprocal(out=recip[:, :nw, :], in_=den_sb[:, :nw, :])
                    for wi, w in enumerate(grp):
                        z = pool.tile([P, HID], dt.float32, tag="z")
                        agg4 = agg[:, wi, :].rearrange("p (h f) -> p h f", h=H)
                        z4 = z[:].rearrange("p (h f) -> p h f", h=H)
                        nc.vector.tensor_tensor(
                            out=z4, in0=agg4,
                            in1=_bcast_last(recip[:, wi, :], cfg.F),
                            op=mybir.AluOpType.mult)
                        res = pool.tile([P, HID], dt.float32, tag="res")
                        nc.vector.scalar_tensor_tensor(
                            out=res[:], in0=z[:], scalar=a_sb[:, 0:1], in1=z[:],
                            op0=mybir.AluOpType.mult, op1=mybir.AluOpType.max)
                        nc.sync.dma_start(out=out[w * P:(w + 1) * P, :], in_=res[:])

    nc.compile()
    return nc


def prepare(cfg: Cfg, inputs):
    """Host-side prep shared by HW and sim paths.

    Returns (sched, in_maps, assemble) where assemble(core_outs) -> full out.
    """
    X = np.asarray(inputs["in_nodes_features"], np.float32)
    ei = np.asarray(inputs["edge_index"], np.int64)
    W = np.asarray(inputs["W"], np.float32)
    b_lin = np.asarray(inputs["b_lin"], np.float32)
    a_src = np.asarray(inputs["a_src"], np.float32)
    a_tgt = np.asarray(inputs["a_tgt"], np.float32)
    bias = np.asarray(inputs["bias"], np.float32)
    prelu_a = float(np.asarray(inputs["prelu_a"], np.float32))

    assert np.all(b_lin == 0) and np.all(bias == 0), "nonzero bias unsupported"
    assert 0.0 <= prelu_a <= 1.0, "prelu_a outside [0,1] unsupported"

    src, trg = ei[0], ei[1]
    core_of = trg // cfg.shard
    win_of = (trg % cfg.shard) // P
    buck_of = src // cfg.bucket
    counts = np.zeros((cfg.ncores, cfg.NW, cfg.NB), np.int64)
    for k in range(cfg.ncores):
        m = core_of == k
        counts[k] = np.bincount(
            win_of[m] * cfg.NB + buck_of[m],
            minlength=cfg.NW * cfg.NB).reshape(cfg.NW, cfg.NB)
    sched = build_schedule(cfg, counts)

    xt = pack_xt(cfg, X)
    wtp, wap = pack_w(cfg, W, a_src, a_tgt)
    av = np.full((P, 1), prelu_a, np.float32)

    in_maps = []
    for k in range(cfg.ncores):
        g1i_k, oh_k = prep_core(cfg, sched, src, trg, k)
        in_maps.append({
            "xt": xt, "wt": wtp, "wa": wap,
            "g1i": g1i_k, "ohd": oh_k, "avec": av,
        })

    def assemble(core_outs):
        return np.concatenate(
            [np.asarray(o["out"][: cfg.shard], np.float32) for o in core_outs], axis=0)

    return sched, in_maps, assemble


_BUILT = {}


def _get_built(cfg: Cfg, sched: Schedule):
    key = (cfg.N, cfg.E, cfg.HID, cfg.HEADS, cfg.ncores, cfg.bucket,
           tuple(sched.TW), sched.idxcols)
    if key not in _BUILT:
        _BUILT[key] = build_nc(cfg, sched)
    return _BUILT[key]


def kernel(**inputs):
    from concourse.bass_utils import run_bass_kernel_spmd

    cfg = Cfg()
    sched, in_maps, assemble = prepare(cfg, inputs)
    nc = _get_built(cfg, sched)
    res = run_bass_kernel_spmd(nc, in_maps, core_ids=list(range(cfg.ncores)))
    return assemble(res.results)



# revision 23
# speedup vs baseline: 1.4232x; 1.1368x over previous
"""GAT layer (multi-head graph attention) on 8 TRN2 NeuronCores.

Strategy (per sharding hint): destination nodes are sharded across the 8
cores.  Each core:
  phase 1: computes the full projection table redundantly (bf16 GEMM
           X @ W.T plus the per-head attention score reductions), packed
           as [proj bf16 | s_src f32 | s_tgt f32 | pad] rows in local HBM.
  phase 2: walks its shard's destination windows (128 targets / window).
           Edges are pre-sorted by (window, src-bucket) on the host;
           dma_gather pulls the source rows (int16 indices per 32768-row
           bucket), scores -> leaky-relu -> exp run batched per window,
           and one-hot matmuls (host-streamed) accumulate both the
           softmax denominator and the weighted aggregation in PSUM.
           Softmax division + PReLU happen once per window at flush.

kernel(**inputs) takes the FULL inputs and returns the FULL output.
"""

import math
from dataclasses import dataclass, field

import numpy as np
import ml_dtypes

BF16 = ml_dtypes.bfloat16
P = 128


def _ceil(a, b):
    return -(-a // b)


@dataclass
class Cfg:
    N: int = 100000
    E: int = 800000
    HID: int = 512
    HEADS: int = 8
    ncores: int = 8
    bucket: int = 32768
    leak: float = 0.01
    oh_bf16: bool = True  # one-hot stream dtype (bf16; fp8 is an option)
    GW: int = 2           # windows per gather group
    CH: int = 16          # tiles per phase-2 compute chunk
    p1_split: bool = True  # split-GEMM phase 1 (interleaved psA/psB)

    def __post_init__(self):
        assert self.N % self.ncores == 0
        assert self.bucket <= 32768
        self.F = self.HID // self.HEADS
        self.shard = self.N // self.ncores
        self.NW = _ceil(self.shard, P)          # windows per core
        self.NB = _ceil(self.N, self.bucket)    # src buckets (int16 range)
        self.NT = _ceil(self.N, P)              # projection tiles
        self.NPAD = self.NT * P
        self.KP = min(self.HID, P)              # contraction partitions
        self.KT = self.HID // self.KP           # contraction tiles
        row_bytes = self.HID * 2 + 2 * self.HEADS * 4
        self.row_used = row_bytes               # bytes actually written
        self.row_bytes = _ceil(row_bytes, 256) * 256
        self.row_bf = self.row_bytes // 2
        self.row_f32 = self.row_bytes // 4
        self.s_src_off = self.HID // 2          # f32 col of s_src in a row
        self.s_tgt_off = self.HID // 2 + self.HEADS
        # phase-1 output split: colsA covers proj[0:splitA] plus the 2H score
        # columns; colsB covers proj[splitA:HID].  Streams are balanced so
        # every LDWEIGHTS hides under the previous matmul's column stream.
        self.splitA = self.HID // 2 - self.HEADS * 2  # 240
        self.colsA = self.splitA + 2 * self.HEADS     # 256
        self.colsB = self.HID - self.splitA           # 272


@dataclass
class Schedule:
    """Core-independent (uniform) phase-2 schedule.

    Windows are processed in groups of GW; each gather call covers one
    (group, bucket) pair so the ~4us fixed per-call GpSimd cost is paid
    ~NW/GW*NB times instead of NW*NB times.  Within a group slots are laid
    out bucket-major: [b0: w0|w1|w2|w3, b1: w0|..., ...], each (w,b)
    segment padded to a 128 multiple so no tile mixes windows.
    """
    seg: np.ndarray          # [NW, NB] slot counts (128-aligned, global max)
    groups: list             # list of lists of window indices
    TG: list                 # tiles per group
    TGmax: int
    calls: list              # per group: list of (b, slot_off, nslots, idxcol0)
    seg_off: dict            # (w, b) -> slot offset within its group
    tile_w: list             # per group: window index (in-group) per tile
    win_last_tile: list      # per group: in-group last tile idx per window
    idxcols: int             # total int16 idx columns (per 16-wrap row)
    TT: int                  # total tiles
    tile_base: list          # first global tile index of each group


def build_schedule(cfg: Cfg, counts: np.ndarray) -> Schedule:
    """counts: [ncores, NW, NB] edge counts."""
    maxcnt = counts.max(axis=0)  # [NW, NB]
    seg = np.where(maxcnt > 0, _ceil(maxcnt, P) * P, 0).astype(np.int64)
    groups = [list(range(g0, min(g0 + cfg.GW, cfg.NW)))
              for g0 in range(0, cfg.NW, cfg.GW)]
    TG, calls, tile_base, tile_w, win_last_tile = [], [], [], [], []
    seg_off = {}
    idxcol = 0
    tt = 0
    for grp in groups:
        tile_base.append(tt)
        gcalls = []
        tw = []
        last = {wi: -1 for wi in range(len(grp))}
        off = 0
        for b in range(cfg.NB):
            nslots = int(sum(seg[w, b] for w in grp))
            if nslots == 0:
                continue
            gcalls.append((b, off, nslots, idxcol))
            for wi, w in enumerate(grp):
                s = int(seg[w, b])
                if s == 0:
                    continue
                seg_off[(w, b)] = off
                for _ in range(s // P):
                    last[wi] = len(tw)
                    tw.append(wi)
                off += s
            idxcol += nslots // 16
        assert off % P == 0
        TG.append(off // P)
        tt += off // P
        calls.append(gcalls)
        tile_w.append(tw)
        win_last_tile.append(last)
    return Schedule(seg=seg, groups=groups, TG=TG, TGmax=max(TG), calls=calls,
                    seg_off=seg_off, tile_w=tile_w, win_last_tile=win_last_tile,
                    idxcols=idxcol, TT=tt, tile_base=tile_base)


def prep_core(cfg: Cfg, sched: Schedule, src, trg, k):
    """Per-core input arrays: g1 idx stream and one-hot stream.

    Padding slots get index 0 (the bucket's first row: real, finite data)
    and all-zero one-hot columns, so they contribute nothing to the
    aggregation or the softmax denominator.
    """
    oh_dt = BF16 if cfg.oh_bf16 else ml_dtypes.float8_e4m3
    mask = (trg // cfg.shard) == k
    esrc = src[mask]
    etrg = trg[mask]
    trel = etrg - k * cfg.shard
    win = trel // P
    buck = esrc // cfg.bucket
    # order edges by (window, bucket); stable so host/device agree
    order = np.lexsort((buck, win))
    esrc, etrg, trel, win, buck = (a[order] for a in (esrc, etrg, trel, win, buck))

    g1i = np.zeros((P, sched.idxcols), np.int16)
    oh = np.zeros((P, sched.TT, 2, P), oh_dt)

    # per (window, bucket) segment boundaries
    key = win * cfg.NB + buck
    starts = np.searchsorted(key, np.arange(cfg.NW * cfg.NB), side="left")
    ends = np.searchsorted(key, np.arange(cfg.NW * cfg.NB), side="right")

    for g, grp in enumerate(sched.groups):
        for (b, call_off, nslots, idxcol0) in sched.calls[g]:
            idx = np.zeros(nslots, np.int16)
            pos = 0
            pad_last = 0
            for w in grp:
                s = int(sched.seg[w, b])
                if s == 0:
                    continue
                lo, hi = int(starts[w * cfg.NB + b]), int(ends[w * cfg.NB + b])
                cnt = hi - lo
                assert cnt <= s
                idx[pos:pos + cnt] = (esrc[lo:hi] - b * cfg.bucket).astype(np.int16)
                # idx[pos+cnt : pos+s] stays 0 (mid-call padding)
                pad_last = s - cnt
                # one-hots for this segment's slots
                tloc = (trel[lo:hi] - w * P).astype(np.int64)   # [cnt] in [0,128)
                gslot = call_off + pos + np.arange(cnt)
                tgl = sched.tile_base[g] + gslot // P
                oh[gslot % P, tgl, 0, tloc] = oh_dt(1.0)
                oh[tloc, tgl, 1, gslot % P] = oh_dt(1.0)
                pos += s
            assert pos == nslots
            # NOTE: trailing -1 indices (ucode-side trim) hang this deployment
            # -- all padding stays index 0 (gathers the bucket's first row).
            del pad_last
            blk = idx.reshape(nslots // 16, 16).T          # [16, cols]
            g1i[:, idxcol0:idxcol0 + nslots // 16] = np.tile(blk, (8, 1))
    return g1i, oh


def pack_xt(cfg: Cfg, X: np.ndarray) -> np.ndarray:
    """X [N, HID] f32 -> bf16 packed [KP, NT, KT, P]: (p, j, ki, n) = X[j*P+n, ki*KP+p]."""
    Xp = np.zeros((cfg.NPAD, cfg.HID), np.float32)
    Xp[: cfg.N] = X
    Xb = Xp.astype(BF16)
    # [NT, P(n), KT, KP(p)] -> transpose to [KP, NT, KT, P]
    v = Xb.reshape(cfg.NT, P, cfg.KT, cfg.KP)
    return np.ascontiguousarray(v.transpose(3, 0, 2, 1))


def pack_w(cfg: Cfg, W, a_src, a_tgt):
    """Returns wA [KP, KT, colsA] and wB [KP, KT, colsB] bf16.

    wA = [W.T[:, :splitA] | WA] (proj prefix plus both score projections),
    wB = W.T[:, splitA:].  Splitting the 528 output columns into two
    balanced streams lets every LDWEIGHTS hide under a matmul stream.
    """
    WT = W.T.astype(np.float32)                       # [HID(d), HID(o)]
    wa_s = (W.reshape(cfg.HEADS, cfg.F, cfg.HID)
            * np.asarray(a_src, np.float32).reshape(cfg.HEADS, cfg.F, 1)).sum(1)  # [H, d]
    wa_t = (W.reshape(cfg.HEADS, cfg.F, cfg.HID)
            * np.asarray(a_tgt, np.float32).reshape(cfg.HEADS, cfg.F, 1)).sum(1)
    WA = np.concatenate([wa_s.T, wa_t.T], axis=1)     # [d, 2H]
    if not cfg.p1_split:
        wAf, wBf = WT, WA      # original layout: full W.T stream + score stream
    else:
        wAf = np.concatenate([WT[:, :cfg.splitA], WA], axis=1)   # [d, colsA]
        wBf = WT[:, cfg.splitA:]                                  # [d, colsB]
    wA = np.ascontiguousarray(
        wAf.astype(BF16).reshape(cfg.KT, cfg.KP, wAf.shape[1]).transpose(1, 0, 2))
    wB = np.ascontiguousarray(
        wBf.astype(BF16).reshape(cfg.KT, cfg.KP, wBf.shape[1]).transpose(1, 0, 2))
    return wA, wB


def _bcast_last(ap, n):
    """Append a 0-stride broadcast dim of size n to an AP."""
    import concourse.bass as bass
    lst = [list(x) for x in ap.ap] + [[0, n]]
    return bass.AP(ap.tensor, ap.offset, lst)


def build_nc(cfg: Cfg, sched: Schedule, phases: str = "full"):
    import concourse.bacc as bacc
    import concourse.bass as bass
    import concourse.mybir as mybir
    from concourse.tile import TileContext

    dt = mybir.dt
    oh_mdt = dt.bfloat16 if cfg.oh_bf16 else dt.float8e4
    H, HID, KT, KP = cfg.HEADS, cfg.HID, cfg.KT, cfg.KP

    nc = bacc.Bacc("TRN2", target_bir_lowering=False)

    cA = cfg.colsA if cfg.p1_split else HID
    cB = cfg.colsB if cfg.p1_split else 2 * H
    xt = nc.dram_tensor("xt", [KP, cfg.NT, KT, P], dt.bfloat16, kind="ExternalInput")
    wt = nc.dram_tensor("wt", [KP, KT, cA], dt.bfloat16, kind="ExternalInput")
    wa = nc.dram_tensor("wa", [KP, KT, cB], dt.bfloat16, kind="ExternalInput")
    g1i = nc.dram_tensor("g1i", [P, sched.idxcols], dt.int16, kind="ExternalInput")
    ohd = nc.dram_tensor("ohd", [P, sched.TT, 2, P], oh_mdt, kind="ExternalInput")
    avec = nc.dram_tensor("avec", [P, 1], dt.float32, kind="ExternalInput")
    out = nc.dram_tensor("out", [cfg.NW * P, HID], dt.float32, kind="ExternalOutput")

    with TileContext(nc) as tc:
        with tc.tile_pool(name="const", bufs=1) as cpool, \
             tc.tile_pool(name="dram", bufs=1, space="DRAM") as dpool:
            table = dpool.tile([cfg.NPAD, cfg.row_bf], dt.bfloat16)
            wt_sb = cpool.tile([KP, KT, cA], dt.bfloat16)
            nc.sync.dma_start(out=wt_sb[:], in_=wt[:, :, :])
            wa_sb = cpool.tile([KP, KT, cB], dt.bfloat16)
            nc.sync.dma_start(out=wa_sb[:], in_=wa[:, :, :])
            if phases == "full":
                a_sb = cpool.tile([P, 1], dt.float32)
                nc.sync.dma_start(out=a_sb[:], in_=avec[:, :])
            if phases in ("full", "p1g"):
                g1i_sb = cpool.tile([P, sched.idxcols], dt.int16)
                nc.sync.dma_start(out=g1i_sb[:], in_=g1i[:, :])

            # ---------------- phase 1: projection table ----------------
            # Two balanced column streams (colsA=256 incl. the 16 score cols,
            # colsB=272) with interleaved matmuls so LDWEIGHTS always hides
            # under the previous stream.  Rows are written 1088B of 1280B
            # (the 192B tail is never read).
            used_bf = cfg.row_used // 2              # 544 bf16 per row
            with tc.tile_pool(name="p1", bufs=3) as xpool, \
                 tc.tile_pool(name="p1ps", bufs=2, space="PSUM") as pspool, \
                 tc.tile_pool(name="p1st", bufs=3) as stpool:
                for j in range(cfg.NT):
                    xtile = xpool.tile([KP, KT, P], dt.bfloat16, tag="x")
                    nc.sync.dma_start(out=xtile[:], in_=xt[:, j, :, :])
                    # full-bank tiles: a matmul output must stay inside one
                    # 2KB PSUM bank, and concurrent accumulation groups must
                    # live in different banks.
                    psA_f = pspool.tile([P, 512], dt.float32, tag="psA")
                    psB_f = pspool.tile([P, 512], dt.float32, tag="psB")
                    psA = psA_f[:, 0:cA]
                    psB = psB_f[:, 0:cB]
                    if cfg.p1_split:
                        for ki in range(KT):
                            nc.tensor.matmul(psA[:], xtile[:, ki, :], wt_sb[:, ki, :],
                                             start=(ki == 0), stop=(ki == KT - 1))
                            nc.tensor.matmul(psB[:], xtile[:, ki, :], wa_sb[:, ki, :],
                                             start=(ki == 0), stop=(ki == KT - 1))
                    else:
                        for ki in range(KT):
                            nc.tensor.matmul(psA[:], xtile[:, ki, :], wt_sb[:, ki, :],
                                             start=(ki == 0), stop=(ki == KT - 1))
                        for ki in range(KT):
                            nc.tensor.matmul(psB[:], xtile[:, ki, :], wa_sb[:, ki, :],
                                             start=(ki == 0), stop=(ki == KT - 1))
                    stg = stpool.tile([P, used_bf], dt.bfloat16, tag="stg")
                    stg32 = stg.bitcast(dt.float32)
                    if cfg.p1_split:
                        nc.scalar.copy(out=stg[:, 0:cfg.splitA],
                                       in_=psA[:, 0:cfg.splitA])
                        nc.scalar.copy(
                            out=stg32[:, cfg.s_src_off:cfg.s_src_off + 2 * H],
                            in_=psA[:, cfg.splitA:cfg.colsA])
                        nc.vector.tensor_copy(out=stg[:, cfg.splitA:HID], in_=psB[:])
                    else:
                        nc.scalar.copy(out=stg[:, 0:HID], in_=psA[:])
                        nc.scalar.copy(
                            out=stg32[:, cfg.s_src_off:cfg.s_src_off + 2 * H],
                            in_=psB[:])
                    nc.sync.dma_start(
                        out=table[j * P:(j + 1) * P, 0:used_bf], in_=stg[:])

            tc.strict_bb_all_engine_barrier()

            # ---------------- phase 1.5: resident s_tgt (hi/lo bf16) ----------------
            pid = nc.sync.partition_id()
            table32 = table.bitcast(dt.float32)
            s_ap = table32[bass.DynSlice(pid * cfg.shard, cfg.NW * P),
                           cfg.s_tgt_off:cfg.s_tgt_off + H]
            s_ap = s_ap.rearrange("(w p) h -> p w h", p=P)
            s_all = cpool.tile([P, cfg.NW, H], dt.float32)
            nc.sync.dma_start(out=s_all[:], in_=s_ap)
            s_hilo = cpool.tile([P, cfg.NW, 2, H], dt.bfloat16)
            s_hi32 = cpool.tile([P, cfg.NW, H], dt.float32)
            nc.vector.tensor_copy(out=s_hilo[:, :, 0, :], in_=s_all[:])
            nc.vector.tensor_copy(out=s_hi32[:], in_=s_hilo[:, :, 0, :])
            nc.vector.tensor_tensor(out=s_hilo[:, :, 1, :], in0=s_all[:],
                                    in1=s_hi32[:], op=mybir.AluOpType.subtract)

            # ---------------- phase 2: window groups ----------------
            CH = cfg.CH
            with tc.tile_pool(name="p2", bufs=2) as pool, \
                 tc.tile_pool(name="p2c", bufs=2) as cpool2, \
                 tc.tile_pool(name="p2ps", bufs=2, space="PSUM") as pps, \
                 tc.tile_pool(name="p2acc", bufs=1, space="PSUM") as apool:
                # Zero both g1t rotation buffers once: slots whose gather was
                # trimmed (trailing -1 indices) read stale SBUF, which must be
                # finite.  After the first two groups, stale bytes are old
                # gathered rows (finite bf16/f32), so one round suffices.
                for _ in range(2):
                    g1z = pool.tile([P, sched.TGmax, cfg.row_bf], dt.bfloat16,
                                    tag="g1t")
                    nc.vector.memset(g1z[:], 0.0)
                for g, grp in enumerate(sched.groups):
                    Tg = sched.TG[g]
                    nw = len(grp)
                    g1t = pool.tile([P, sched.TGmax, cfg.row_bf], dt.bfloat16,
                                    tag="g1t")
                    for (b, slot_off, nslots, idxcol0) in sched.calls[g]:
                        rows = min(cfg.NPAD, (b + 1) * cfg.bucket) - b * cfg.bucket
                        # single_packet chains the call's descriptors into
                        # one SDMA packet; the HW packet limit is 64
                        # descriptors, so large merged calls must split.
                        nc.gpsimd.dma_gather(
                            g1t[:, slot_off // P:(slot_off + nslots) // P, :],
                            table[b * cfg.bucket:b * cfg.bucket + rows, :],
                            g1i_sb[:, idxcol0:idxcol0 + nslots // 16],
                            nslots, nslots, cfg.row_bf,
                            single_packet=(nslots // 16 + 1 <= 64))
                    g1t32 = g1t.bitcast(dt.float32)
                    jb = sched.tile_base[g]
                    agg = apool.tile([P, cfg.GW, HID], dt.float32, tag="agg")
                    den = apool.tile([P, cfg.GW, H], dt.float32, tag="den")
                    den_acc = pool.tile([P, cfg.GW, H], dt.float32, tag="den_acc")
                    den_seen = set()
                    started = [False] * nw
                    for c in range(_ceil(Tg, CH)):
                        t0, t1 = c * CH, min(Tg, (c + 1) * CH)
                        tn = t1 - t0
                        ohc = cpool2.tile([P, CH, 2, P], oh_mdt, tag="ohc")
                        nc.sync.dma_start(out=ohc[:, :tn, :, :],
                                          in_=ohd[:, jb + t0:jb + t1, :, :])
                        stgt = pps.tile([P, CH, 2, H], dt.float32, tag="stgt")
                        for t in range(t0, t1):
                            nc.tensor.matmul(
                                stgt[:, t - t0, :, :], ohc[:, t - t0, 1, :],
                                s_hilo[:, grp[sched.tile_w[g][t]], :, :],
                                start=True, stop=True)
                        s_sum = cpool2.tile([P, CH, H], dt.float32, tag="s_sum")
                        s_act = cpool2.tile([P, CH, H], dt.float32, tag="s_act")
                        nc.vector.tensor_tensor(
                            out=s_sum[:, :tn, :], in0=stgt[:, :tn, 0, :],
                            in1=g1t32[:, t0:t1, cfg.s_src_off:cfg.s_src_off + H],
                            op=mybir.AluOpType.add)
                        nc.vector.tensor_tensor(
                            out=s_act[:, :tn, :], in0=stgt[:, :tn, 1, :],
                            in1=s_sum[:, :tn, :], op=mybir.AluOpType.add)
                        nc.vector.scalar_tensor_tensor(
                            out=s_sum[:, :tn, :], in0=s_act[:, :tn, :],
                            scalar=cfg.leak, in1=s_act[:, :tn, :],
                            op0=mybir.AluOpType.mult, op1=mybir.AluOpType.max)
                        exp_t = cpool2.tile([P, CH, H], dt.bfloat16, tag="exp_t")
                        nc.scalar.activation(out=exp_t[:, :tn, :],
                                             in_=s_sum[:, :tn, :],
                                             func=mybir.ActivationFunctionType.Exp)

                        w_t = cpool2.tile([P, CH, HID], dt.bfloat16, tag="w_t")
                        proj4 = g1t[:, t0:t1, 0:HID].rearrange(
                            "p t (h f) -> p t h f", h=H)
                        exp4 = _bcast_last(exp_t[:, :tn, :], cfg.F)
                        out4 = w_t[:, :tn, :].rearrange("p t (h f) -> p t h f", h=H)
                        nc.vector.tensor_tensor(out=out4, in0=proj4, in1=exp4,
                                                op=mybir.AluOpType.mult)

                        # tiles grouped by window: agg chains span the whole
                        # group (each window's agg is its own PSUM bank); den
                        # groups open/close within this chunk (all windows
                        # share one bank, so groups must not interleave), and
                        # chunk partials accumulate into den_acc on DVE.
                        bywin = {}
                        for t in range(t0, t1):
                            bywin.setdefault(sched.tile_w[g][t], []).append(t)
                        for wi, tlist in bywin.items():
                            for t in tlist:
                                first = not started[wi]
                                started[wi] = True
                                lastt = (t == sched.win_last_tile[g][wi])
                                nc.tensor.matmul(agg[:, wi, :],
                                                 ohc[:, t - t0, 0, :],
                                                 w_t[:, t - t0, :],
                                                 start=first, stop=lastt)
                                nc.tensor.matmul(den[:, wi, :],
                                                 ohc[:, t - t0, 0, :],
                                                 exp_t[:, t - t0, :],
                                                 start=(t == tlist[0]),
                                                 stop=(t == tlist[-1]))
                            if wi in den_seen:
                                nc.vector.tensor_tensor(
                                    out=den_acc[:, wi, :], in0=den[:, wi, :],
                                    in1=den_acc[:, wi, :], op=mybir.AluOpType.add)
                            else:
                                nc.vector.tensor_copy(out=den_acc[:, wi, :],
                                                      in_=den[:, wi, :])
                                den_seen.add(wi)

                    # flush: softmax divide + PReLU for all windows in group
                    den_sb = pool.tile([P, cfg.GW, H], dt.float32, tag="den_sb")
                    recip = pool.tile([P, cfg.GW, H], dt.float32, tag="recip")
                    nc.vector.tensor_scalar_add(out=den_sb[:, :nw, :],
                                                in0=den_acc[:, :nw, :],
                                                scalar1=1e-16)
                    nc.vector.reciprocal(out=recip[:, :nw, :], in_=den_sb[:, :nw, :])
                    for wi, w in enumerate(grp):
                        z = pool.tile([P, HID], dt.float32, tag="z")
                        agg4 = agg[:, wi, :].rearrange("p (h f) -> p h f", h=H)
                        z4 = z[:].rearrange("p (h f) -> p h f", h=H)
                        nc.vector.tensor_tensor(
                            out=z4, in0=agg4,
                            in1=_bcast_last(recip[:, wi, :], cfg.F),
                            op=mybir.AluOpType.mult)
                        res = pool.tile([P, HID], dt.float32, tag="res")
                        nc.vector.scalar_tensor_tensor(
                            out=res[:], in0=z[:], scalar=a_sb[:, 0:1], in1=z[:],
                            op0=mybir.AluOpType.mult, op1=mybir.AluOpType.max)
                        nc.sync.dma_start(out=out[w * P:(w + 1) * P, :], in_=res[:])

    nc.compile()
    return nc


def prepare(cfg: Cfg, inputs):
    """Host-side prep shared by HW and sim paths.

    Returns (sched, in_maps, assemble) where assemble(core_outs) -> full out.
    """
    X = np.asarray(inputs["in_nodes_features"], np.float32)
    ei = np.asarray(inputs["edge_index"], np.int64)
    W = np.asarray(inputs["W"], np.float32)
    b_lin = np.asarray(inputs["b_lin"], np.float32)
    a_src = np.asarray(inputs["a_src"], np.float32)
    a_tgt = np.asarray(inputs["a_tgt"], np.float32)
    bias = np.asarray(inputs["bias"], np.float32)
    prelu_a = float(np.asarray(inputs["prelu_a"], np.float32))

    assert np.all(b_lin == 0) and np.all(bias == 0), "nonzero bias unsupported"
    assert 0.0 <= prelu_a <= 1.0, "prelu_a outside [0,1] unsupported"

    src, trg = ei[0], ei[1]
    core_of = trg // cfg.shard
    win_of = (trg % cfg.shard) // P
    buck_of = src // cfg.bucket
    counts = np.zeros((cfg.ncores, cfg.NW, cfg.NB), np.int64)
    for k in range(cfg.ncores):
        m = core_of == k
        counts[k] = np.bincount(
            win_of[m] * cfg.NB + buck_of[m],
            minlength=cfg.NW * cfg.NB).reshape(cfg.NW, cfg.NB)
    sched = build_schedule(cfg, counts)

    xt = pack_xt(cfg, X)
    wtp, wap = pack_w(cfg, W, a_src, a_tgt)
    av = np.full((P, 1), prelu_a, np.float32)

    in_maps = []
    for k in range(cfg.ncores):
        g1i_k, oh_k = prep_core(cfg, sched, src, trg, k)
        in_maps.append({
            "xt": xt, "wt": wtp, "wa": wap,
            "g1i": g1i_k, "ohd": oh_k, "avec": av,
        })

    def assemble(core_outs):
        return np.concatenate(
            [np.asarray(o["out"][: cfg.shard], np.float32) for o in core_outs], axis=0)

    return sched, in_maps, assemble


_BUILT = {}


def _get_built(cfg: Cfg, sched: Schedule):
    key = (cfg.N, cfg.E, cfg.HID, cfg.HEADS, cfg.ncores, cfg.bucket,
           tuple(sched.TW), sched.idxcols)
    if key not in _BUILT:
        _BUILT[key] = build_nc(cfg, sched)
    return _BUILT[key]


def kernel(**inputs):
    from concourse.bass_utils import run_bass_kernel_spmd

    cfg = Cfg()
    sched, in_maps, assemble = prepare(cfg, inputs)
    nc = _get_built(cfg, sched)
    res = run_bass_kernel_spmd(nc, in_maps, core_ids=list(range(cfg.ncores)))
    return assemble(res.results)

